# revision 21
# baseline (speedup 1.0000x reference)
"""Trainium2 Bass kernel for nn_BlockMoE (attention + top-2 MoE block), 8-core SPMD.

Sharding: attention is T-sharded (core c handles batch c//4, a 512-token chunk
with a 1024-token KV halo); MoE is expert-parallel (core e owns expert e).
Cross-core exchange: AllGather of per-core expert counts, AllGather of xn1
(bf16), ReduceScatter of the dispatch table.  Host only shards inputs /
gathers and sums outputs.
"""
import sys

for _p in ("/opt/trn_rl_repo",):
    if _p not in sys.path:
        sys.path.insert(0, _p)

import numpy as np
import ml_dtypes

import concourse.bass as bass
import concourse.mybir as mybir
import bass_rust as _bass_rust
from concourse.tile import TileContext
from concourse.masks import make_identity
from concourse.bass_utils import run_bass_kernel_spmd

F32 = mybir.dt.float32
F32R = mybir.dt.float32r
BF16 = mybir.dt.bfloat16
I32 = mybir.dt.int32
U32 = mybir.dt.uint32
AF = mybir.ActivationFunctionType
OP = mybir.AluOpType
AX = mybir.AxisListType

P = 128
B, T, C = 2, 2048, 1024
NH, NKV, HD = 16, 8, 64
E, H = 8, 2048
NTOK = B * T
TQ = 512
TK = 1536
KT = TK // P       # 12
QT = TQ // P       # 4
CT = C // P        # 8
HT = H // P        # 16
CAP = 2 * NTOK * 2 // E   # 2048
EPS = float(np.finfo(np.float32).eps)
NCORES = 8
QK_DT = BF16       # dtype for q^T/k^T storage + scores matmul


def _split_waits(nc, max_waits=1):
    """This walrus accepts at most 1 sem-wait per instruction; hoist the rest
    onto nops inserted just before."""
    n_fixed = 0
    for bb in nc.main_func.blocks:
        targets = [
            ins
            for ins in bb.instructions
            if ins.sync_info is not None
            and ins.sync_info.on_wait
            and len(ins.sync_info.on_wait) > max_waits
            and ins.engine != mybir.EngineType.Unassigned
        ]
        for ins in targets:
            waits = list(ins.sync_info.on_wait)
            keep, rest = waits[:max_waits], waits[max_waits:]
            nops = []
            for i in range(0, len(rest), max_waits):
                chunk = rest[i : i + max_waits]
                bi = nc.engines[ins.engine].nop(nofuse=True)
                nop_inst = bi.ins
                for bb2 in nc.main_func.blocks:
                    if nop_inst in bb2.instructions:
                        bb2.instructions.remove(nop_inst)
                nop_inst.sync_info = _bass_rust.SyncInfo(on_wait=chunk, on_update=[])
                nops.append(nop_inst)
                n_fixed += 1
            ins.sync_info = _bass_rust.SyncInfo(
                on_wait=keep, on_update=list(ins.sync_info.on_update or [])
            )
            pos = bb.instructions.index(ins)
            bb.instructions[pos:pos] = nops
    return n_fixed


def _build():
    nc = bass.Bass("TRN2", target_bir_lowering=False)
    dp = nc.declare_dram_parameter
    io = {}
    io["xT_in"] = dp("xT", [C, TK], F32R, isOutput=False)
    io["xq_in"] = dp("xq", [TQ, C], F32, isOutput=False)
    io["ve_in"] = dp("ve", [TK, NKV * HD], F32, isOutput=False)
    io["cosk_in"] = dp("cosk", [TK, 32], F32, isOutput=False)
    io["sink_in"] = dp("sink", [TK, 32], F32, isOutput=False)
    io["padb_in"] = dp("padb", [TK, 1], F32, isOutput=False)
    io["trimask_in"] = dp("trimask", [8 * P, TQ], BF16, isOutput=False)
    io["wqT_in"] = dp("wqT", [C, NH * HD], F32R, isOutput=False)
    io["wkT_in"] = dp("wkT", [C, NKV * HD], F32R, isOutput=False)
    io["wvT_in"] = dp("wvT", [C, NKV * HD], F32R, isOutput=False)
    io["gateT_in"] = dp("gateT", [32, 8], F32R, isOutput=False)
    io["woT_in"] = dp("woT", [C, C], BF16, isOutput=False)
    io["routerT_in"] = dp("routerT", [C, E], F32, isOutput=False)
    io["fcT_in"] = dp("fcT", [C, H], BF16, isOutput=False)
    io["projT_in"] = dp("projT", [H, C], BF16, isOutput=False)
    io["corelt_in"] = dp("corelt", [8, 1], F32, isOutput=False)
    io["e2048_in"] = dp("e2048", [8, 1], F32, isOutput=False)
    io["iota8_in"] = dp("iota8", [P, 8], F32, isOutput=False)
    io["tokp1_in"] = dp("tokp1", [P, 8], F32, isOutput=False)
    io["x1_out"] = dp("x1o", [TQ, C], F32, isOutput=True)
    io["rw_out"] = dp("rwo", [TQ, E], F32, isOutput=True)
    io["moe_out"] = dp("moeo", [NTOK, C], F32, isOutput=True)

    with TileContext(nc) as tc:
        _program(nc, tc, io)
    _split_waits(nc)
    return nc


def _program(nc, tc, io):
    import contextlib

    ctx = contextlib.ExitStack()
    with ctx:
        const = ctx.enter_context(tc.tile_pool(name="const", bufs=1))
        wpool = ctx.enter_context(tc.tile_pool(name="wpool", bufs=1))
        spool = ctx.enter_context(tc.tile_pool(name="spool", bufs=2))
        hold = ctx.enter_context(tc.tile_pool(name="hold", bufs=1))
        psA = ctx.enter_context(tc.tile_pool(name="psA", bufs=2, space="PSUM"))
        psB = ctx.enter_context(tc.tile_pool(name="psB", bufs=3, space="PSUM"))
        psY = ctx.enter_context(tc.tile_pool(name="psY", bufs=1, space="PSUM"))

        _ctr = [0]

        def pa():
            _ctr[0] += 1
            return psA.tile([P, 1024], F32, tag="A", name=f"psA_{_ctr[0]}")

        def pb(shape=None):
            _ctr[0] += 1
            return psB.tile([P, 512], F32, tag="B", name=f"psB_{_ctr[0]}")

        def pbb():
            _ctr[0] += 1
            return psB.tile([P, 512], BF16, tag="B", name=f"psBb_{_ctr[0]}")

        dram = ctx.enter_context(tc.tile_pool(name="dram", bufs=1, space="DRAM"))

        # ===== constants =====
        ident = const.tile([P, P], F32)
        make_identity(nc, ident[:])
        identb = const.tile([P, P], BF16)
        nc.vector.tensor_copy(identb[:], ident[:])
        ones1b = const.tile([P, 1], BF16)
        nc.vector.memset(ones1b[:], 1.0)
        ones8 = const.tile([8, 1], F32)
        nc.vector.memset(ones8[:], 1.0)
        onesb = const.tile([P, HD], BF16)
        nc.vector.memset(onesb[:], 1.0)
        padb = const.tile([P, KT], F32)
        nc.sync.dma_start(out=padb[:].rearrange("p (a b) -> p a b", b=1), in_=io["padb_in"][:].rearrange("(a p) b -> p a b", p=P))
        iota8 = const.tile([P, 8], F32)
        nc.sync.dma_start(out=iota8[:], in_=io["iota8_in"][:])
        tokp1 = const.tile([P, 8], F32)
        nc.sync.dma_start(out=tokp1[:], in_=io["tokp1_in"][:])
        corelt = const.tile([8, 1], F32)
        nc.sync.dma_start(out=corelt[:], in_=io["corelt_in"][:])
        epsc = const.tile([P, 1], F32)
        nc.vector.memset(epsc[:], EPS)
        e2048 = const.tile([8, 1], F32)
        nc.sync.dma_start(out=e2048[:], in_=io["e2048_in"][:])
        gateT = const.tile([32, 8], F32R)
        nc.sync.dma_start(out=gateT[:], in_=io["gateT_in"][:])
        routerT = const.tile([P, CT * E], F32)
        nc.sync.dma_start(
            out=routerT[:].rearrange("p (a e) -> p a e", e=E), in_=io["routerT_in"][:].rearrange("(a p) e -> p a e", p=P)
        )
        masks = []
        for i in range(8):
            mt = const.tile([P, TQ], BF16, tag=f"mask{i}")
            nc.sync.dma_start(out=mt[:], in_=io["trimask_in"][i * P : (i + 1) * P, :])
            masks.append(mt)
        cosk, sink = [], []
        for tt in range(KT):
            ctile = const.tile([P, 32], F32, tag=f"cos{tt}")
            stile = const.tile([P, 32], F32, tag=f"sin{tt}")
            nc.sync.dma_start(out=ctile[:], in_=io["cosk_in"][tt * P : (tt + 1) * P, :])
            nc.sync.dma_start(out=stile[:], in_=io["sink_in"][tt * P : (tt + 1) * P, :])
            cosk.append(ctile)
            sink.append(stile)

        # resident attention out (transposed q/k spilled to DRAM)
        yT = hold.tile([HD, NH * TQ], BF16)
        rcol = hold.tile([P, KT], F32)
        S = hold.tile([8, TQ * 2], BF16)

        # DRAM scratch
        kT_d = dram.tile([HD, NKV * TK], QK_DT)
        vext_d = dram.tile([NKV * TK, HD + 1], BF16)
        qT_d = dram.tile([HD, NH * TQ], QK_DT)
        xn1b = dram.tile([TQ, C], BF16)
        agx = dram.tile([NTOK, C], BF16, addr_space="Shared")
        cntb = dram.tile([8, 1], F32)
        cntag = dram.tile([NCORES * 8, 1], F32, addr_space="Shared")
        wtmp = dram.tile([TQ * 2, 1], F32)
        ttmp = dram.tile([TQ * 2, 1], F32)
        table = dram.tile([E * CAP, 2], F32)
        tabrs = dram.tile([CAP, 2], F32)

        # weight slots (reused across passes)
        def load_w(name, cols, dtype, tagp):
            tiles = []
            for i in range(CT):
                t = wpool.tile([P, cols], dtype, tag=f"{tagp}{i}")
                nc.sync.dma_start(out=t[:], in_=io[name][i * P : (i + 1) * P, :])
                tiles.append(t)
            return tiles

        def stream_x(tt, tag="xs"):
            xs = []
            for ct in range(CT):
                t = spool.tile([P, P], F32R, tag=f"{tag}{ct}", bufs=2)
                nc.sync.dma_start(
                    out=t[:], in_=io["xT_in"][ct * P : (ct + 1) * P, tt * P : (tt + 1) * P]
                )
                xs.append(t)
            return xs

        def rope_norm(nc, ps_raw, nh, tt, dst, dst_col):
            """psum raw [P, nh*HD] -> rope -> per-head rms -> transpose into
            dst[:, head*W + dst_col*P : ...] (dst width W per head)."""
            rc = rcol[:, tt : tt + 1]
            raw = spool.tile([P, 16, HD], F32, tag="rp_raw")
            r3 = raw[:, 0:nh, :]
            nc.vector.tensor_scalar_mul(r3, ps_raw[:].rearrange("p (h d) -> p h d", h=nh), rc)
            cb = cosk[tt][:].rearrange("p (h d) -> p h d", h=1).to_broadcast([P, nh, 32])
            sb = sink[tt][:].rearrange("p (h d) -> p h d", h=1).to_broadcast([P, nh, 32])
            m1 = spool.tile([P, 16, 32], F32, tag="rp_m1", bufs=1)
            m2 = spool.tile([P, 16, 32], F32, tag="rp_m2", bufs=1)
            hat = spool.tile([P, 16, HD], F32, tag="rp_hn", bufs=1)
            h3 = hat[:, 0:nh, :]
            nc.vector.tensor_tensor(out=m1[:, 0:nh], in0=r3[:, :, 0:32], in1=cb, op=OP.mult)
            nc.vector.tensor_tensor(out=m2[:, 0:nh], in0=r3[:, :, 32:64], in1=sb, op=OP.mult)
            nc.vector.tensor_tensor(out=h3[:, :, 0:32], in0=m1[:, 0:nh], in1=m2[:, 0:nh], op=OP.add)
            nc.vector.tensor_tensor(out=m1[:, 0:nh], in0=r3[:, :, 32:64], in1=cb, op=OP.mult)
            nc.vector.tensor_tensor(out=m2[:, 0:nh], in0=r3[:, :, 0:32], in1=sb, op=OP.mult)
            nc.vector.tensor_tensor(out=h3[:, :, 32:64], in0=m1[:, 0:nh], in1=m2[:, 0:nh], op=OP.subtract)
            sq = spool.tile([P, 16, HD], F32, tag="rp_raw", name="rp_sqv")
            nc.vector.tensor_tensor(out=sq[:, 0:nh], in0=h3, in1=h3, op=OP.mult)
            ssum = spool.tile([P, 16], F32, tag="rp_ss")
            nc.vector.tensor_reduce(ssum[:, 0:nh], sq[:, 0:nh], axis=AX.X, op=OP.add)
            rh = spool.tile([P, 16], F32, tag="rp_rh")
            nc.scalar.activation(rh[:, 0:nh], ssum[:, 0:nh], AF.Sqrt, bias=epsc[:], scale=1.0 / HD)
            nc.vector.reciprocal(rh[:, 0:nh], rh[:, 0:nh])
            nc.vector.tensor_tensor(
                out=h3, in0=h3, in1=rh[:, 0:nh].to_broadcast([P, nh, HD]), op=OP.mult
            )
            asm = spool.tile([HD, 16 * P], QK_DT, tag="rp_asm")
            for h in range(nh):
                pt = pb()
                nc.tensor.transpose(pt[0:HD, 0:P], hat[:, h, :], ident[:])
                nc.vector.tensor_copy(asm[:, h * P : (h + 1) * P], pt[0:HD, 0:P])
            nc.sync.dma_start(
                out=dst[:].rearrange("a (h w) -> a h w", h=nh)[:, :, dst_col * P : (dst_col + 1) * P],
                in_=asm[:, 0 : nh * P].rearrange("a (h w) -> a h w", h=nh),
            )

        # ================= pass K (+ rms1 fused) =================
        wk = load_w("wkT_in", NKV * HD, F32R, "ws")
        for tt in range(KT):
            xs = stream_x(tt)
            # rms1 for this token tile
            ss = pb()
            for ct in range(CT):
                sq = spool.tile([P, P], BF16, tag="sq1", bufs=2)
                nc.scalar.activation(sq[:], xs[ct][:].bitcast(F32), AF.Square)
                nc.tensor.matmul(ss[0:1, 0:P], ones1b[:], sq[:], start=(ct == 0), stop=(ct == CT - 1))
            sq1 = spool.tile([1, P], F32, tag="sq1r", bufs=2)
            nc.vector.tensor_copy(sq1[:], ss[0:1, 0:P])
            pt = pb()
            nc.tensor.transpose(pt[0:P, 0:1], sq1[:], ident[0:1, 0:1])
            nc.scalar.activation(rcol[:, tt : tt + 1], pt[0:P, 0:1], AF.Sqrt, bias=epsc[:], scale=1.0 / C)
            nc.vector.reciprocal(rcol[:, tt : tt + 1], rcol[:, tt : tt + 1])
            # k
            ps_k = pb()
            for ct in range(CT):
                nc.tensor.matmul(ps_k[:], xs[ct][:], wk[ct][:], start=(ct == 0), stop=(ct == CT - 1))
            rope_norm(nc, ps_k, NKV, tt, kT_d, tt)

        # ================= pass V (+ gate) =================
        wv = load_w("wvT_in", NKV * HD, F32R, "ws")
        for tt in range(KT):
            xs = stream_x(tt)
            rc = rcol[:, tt : tt + 1]
            ps_v = pb()
            for ct in range(CT):
                nc.tensor.matmul(ps_v[:], xs[ct][:], wv[ct][:], start=(ct == 0), stop=(ct == CT - 1))
            ps_g = pb()
            nc.tensor.matmul(ps_g[0:P, 0:8], xs[0][0:32, :], gateT[:], start=True, stop=True)
            ga = spool.tile([P, NKV], F32, tag="ga")
            nc.scalar.activation(ga[:], ps_g[0:P, 0:8], AF.Sigmoid, scale=rc)
            vtmp = spool.tile([P, NKV * HD], F32, tag="vtmp")
            nc.vector.tensor_scalar_mul(vtmp[:], ps_v[:], rc)
            vet = spool.tile([P, NKV * HD], F32, tag="vet", bufs=2)
            nc.sync.dma_start(out=vet[:], in_=io["ve_in"][tt * P : (tt + 1) * P, :])
            nc.vector.tensor_tensor(
                out=vet[:].rearrange("p (h d) -> p h d", h=NKV),
                in0=vet[:].rearrange("p (h d) -> p h d", h=NKV),
                in1=ga[:].to_broadcast([P, NKV, HD]),
                op=OP.mult,
            )
            nc.vector.scalar_tensor_tensor(
                out=vtmp[:], in0=vet[:], scalar=2.0, in1=vtmp[:], op0=OP.mult, op1=OP.add
            )
            vtile = spool.tile([P, NKV, HD + 1], BF16, tag="vtile", bufs=2)
            nc.vector.tensor_copy(vtile[:, :, 0:HD], vtmp[:].rearrange("p (h d) -> p h d", h=NKV))
            nc.vector.memset(vtile[:, :, HD : HD + 1], 1.0)
            nc.sync.dma_start(
                out=vext_d[:].rearrange("(h r) d -> h r d", h=NKV)[
                    :, tt * P : (tt + 1) * P, :
                ].rearrange("h p d -> p h d"),
                in_=vtile[:],
            )

        # ================= pass Q =================
        wq = load_w("wqT_in", NH * HD, F32R, "ws")
        for tt in range(8, KT):
            xs = stream_x(tt)
            ps_q = pa()
            for half in range(2):
                for ct in range(CT):
                    nc.tensor.matmul(
                        ps_q[:, half * 512 : (half + 1) * 512],
                        xs[ct][:],
                        wq[ct][:, half * 512 : (half + 1) * 512],
                        start=(ct == 0),
                        stop=(ct == CT - 1),
                    )
            rope_norm(nc, ps_q, NH, tt, qT_d, tt - 8)

        # ================= A4: scores/softmax/pv =================
        yTv = yT[:].rearrange("a (h w) -> a h w", h=NH)
        for h in range(NKV):
            vexth = spool.tile([P, KT, HD + 1], BF16, tag="vexth", bufs=2)
            nc.sync.dma_start(
                out=vexth[:],
                in_=vext_d[:].rearrange("(h r) d -> h r d", h=NKV)[h, :, :].rearrange(
                    "(t p) d -> p t d", p=P
                ),
            )
            kTh = spool.tile([HD, TK], QK_DT, tag="kTh", bufs=2)
            nc.sync.dma_start(out=kTh[:], in_=kT_d[:].rearrange("a (h w) -> a h w", h=NKV)[:, h, :])
            for qh in (2 * h, 2 * h + 1):
                qTh = spool.tile([HD, TQ], QK_DT, tag="qTh", bufs=2)
                nc.sync.dma_start(out=qTh[:], in_=qT_d[:].rearrange("a (h w) -> a h w", h=NH)[:, qh, :])
                ps_y = psY.tile([HD + 1, TQ], F32, tag="Y")
                for kt in range(KT):
                    ps_s = pb()
                    nc.tensor.matmul(
                        ps_s[:], kTh[:, kt * P : (kt + 1) * P], qTh[:],
                        start=True, stop=True,
                    )
                    pT = spool.tile([P, TQ], BF16, tag="pT", bufs=2)
                    nc.scalar.activation(
                        pT[:], ps_s[:], AF.Exp, bias=padb[:, kt : kt + 1], scale=0.125
                    )
                    if kt < 4:
                        nc.vector.tensor_tensor(out=pT[:], in0=pT[:], in1=masks[kt][:], op=OP.mult)
                    elif kt >= 8:
                        nc.vector.tensor_tensor(out=pT[:], in0=pT[:], in1=masks[kt - 4][:], op=OP.mult)
                    nc.tensor.matmul(
                        ps_y[:], vexth[:, kt, :], pT[:], start=(kt == 0), stop=(kt == KT - 1)
                    )
                zrow = spool.tile([P, TQ], F32, tag="vtmp")
                nc.vector.reciprocal(zrow[HD : HD + 1, :], ps_y[HD : HD + 1, :])
                bcs = spool.tile([P, TQ], BF16, tag="bcs")
                nc.vector.tensor_copy(bcs[HD : HD + 1, :], zrow[HD : HD + 1, :])
                ps_b = pb()
                nc.tensor.matmul(
                    ps_b[0:HD, :], onesb[HD : HD + 1, :], bcs[HD : HD + 1, :],
                    start=True, stop=True,
                )
                nc.vector.tensor_copy(bcs[0:HD, :], ps_b[0:HD, :])
                nc.vector.tensor_tensor(
                    out=yTv[:, qh, :], in0=ps_y[0:HD, :], in1=bcs[0:HD, :], op=OP.mult
                )

        # ================= A5/A6: proj+residual+rms2+router =================
        woT = []
        for i in range(NH):
            wt = wpool.tile([HD, C], BF16, tag=f"wo{i}", name=f"woT{i}")
            nc.sync.dma_start(out=wt[:], in_=io["woT_in"][i * HD : (i + 1) * HD, :])
            woT.append(wt)
        for m in range(QT):
            ps_o = pa()
            for n in range(2):
                for qh in range(NH):
                    nc.tensor.matmul(
                        ps_o[:, n * 512 : (n + 1) * 512],
                        yTv[:, qh, m * P : (m + 1) * P],
                        woT[qh][:, n * 512 : (n + 1) * 512],
                        start=(qh == 0),
                        stop=(qh == NH - 1),
                    )
            xqt = spool.tile([P, C], F32, tag="rp_raw", bufs=2)
            nc.sync.dma_start(out=xqt[:], in_=io["xq_in"][m * P : (m + 1) * P, :])
            x1m = spool.tile([P, C], F32, tag="x1m", bufs=1)
            nc.vector.tensor_tensor(out=x1m[:], in0=ps_o[:], in1=xqt[:], op=OP.add)
            nc.sync.dma_start(out=io["x1_out"][m * P : (m + 1) * P, :], in_=x1m[:])
            sq = spool.tile([P, C], F32, tag="rp_hn", bufs=1)
            nc.vector.tensor_tensor(out=sq[:], in0=x1m[:], in1=x1m[:], op=OP.mult)
            ssum = spool.tile([P, 1], F32, tag="ss_a6")
            nc.vector.tensor_reduce(ssum[:], sq[:], axis=AX.X, op=OP.add)
            r1 = spool.tile([P, 1], F32, tag="r1_a6")
            nc.scalar.activation(r1[:], ssum[:], AF.Sqrt, bias=epsc[:], scale=1.0 / C)
            nc.vector.reciprocal(r1[:], r1[:])
            xn1 = spool.tile([P, C], F32, tag="rp_sq2", bufs=2)
            nc.vector.tensor_scalar_mul(xn1[:], x1m[:], r1[:])
            xn1c = spool.tile([P, C], BF16, tag="xn1c", bufs=1)
            nc.vector.tensor_copy(xn1c[:], xn1[:])
            nc.sync.dma_start(out=xn1b[m * P : (m + 1) * P, :], in_=xn1c[:])
            # router (plain f32 matmuls, exact)
            ps_r = pb()
            for ct in range(CT):
                ptr = pb()
                nc.tensor.transpose(ptr[:, 0:P], xn1[:, ct * P : (ct + 1) * P], ident[:])
                xn1T = spool.tile([P, P], F32, tag="xn1T", bufs=2)
                nc.vector.tensor_copy(xn1T[:], ptr[:, 0:P])
                nc.tensor.matmul(
                    ps_r[0:P, 0:E], xn1T[:], routerT[:, ct * E : (ct + 1) * E],
                    start=(ct == 0), stop=(ct == CT - 1),
                )
            er = spool.tile([P, E], F32, tag="er")
            nc.scalar.activation(er[:], ps_r[0:P, 0:E], AF.Exp)
            es = spool.tile([P, 1], F32, tag="es")
            nc.vector.tensor_reduce(es[:], er[:], axis=AX.X, op=OP.add)
            nc.vector.reciprocal(es[:], es[:])
            rw = spool.tile([P, E], F32, tag="rw")
            nc.vector.tensor_scalar_mul(rw[:], er[:], es[:])
            nc.sync.dma_start(out=io["rw_out"][m * P : (m + 1) * P, :], in_=rw[:])
            tv = spool.tile([P, 8], F32, tag="tv")
            ti = spool.tile([P, 8], U32, tag="ti")
            nc.vector.max_with_indices(tv[:], ti[:], rw[:])
            tif = spool.tile([P, 8], F32, tag="tif")
            nc.vector.tensor_copy(tif[:], ti[:])
            den = spool.tile([P, 1], F32, tag="den")
            nc.vector.tensor_tensor(out=den[:], in0=tv[:, 0:1], in1=tv[:, 1:2], op=OP.add)
            nc.vector.tensor_scalar_add(den[:], den[:], 1e-10)
            nc.vector.reciprocal(den[:], den[:])
            swp = spool.tile([P, 2], F32, tag="swp")
            nc.vector.tensor_scalar_mul(swp[:], tv[:, 0:2], den[:])
            nc.sync.dma_start(
                out=wtmp[m * 256 : (m + 1) * 256, :].rearrange("(p c) b -> p c b", c=2),
                in_=swp[:].rearrange("p (c b) -> p c b", b=1),
            )
            for k in range(2):
                oh = spool.tile([P, 8], F32, tag="oh")
                nc.vector.tensor_tensor(
                    out=oh[:], in0=tif[:, k : k + 1].to_broadcast([P, 8]), in1=iota8[:],
                    op=OP.is_equal,
                )
                pso = pb()
                nc.tensor.transpose(pso[0:8, 0:P], oh[:], ident[:])
                nc.vector.tensor_copy(
                    S[:].rearrange("e (x c) -> e x c", c=2)[:, m * P : (m + 1) * P, k],
                    pso[0:8, 0:P],
                )

        # ================= A7: routing =================
        zc8 = const.tile([8, 1], F32)
        nc.vector.memset(zc8[:], 0.0)
        incl = hold.tile([8, TQ * 2], F32)
        nc.vector.tensor_tensor_scan(
            incl[:], S[:], zc8[:].to_broadcast([8, TQ * 2]), 0.0, op0=OP.add, op1=OP.add
        )
        cnt = spool.tile([8, 1], F32, tag="cnt")
        nc.vector.tensor_copy(cnt[:], incl[:, TQ * 2 - 1 : TQ * 2])
        nc.vector.tensor_tensor(out=incl[:], in0=incl[:], in1=S[:], op=OP.subtract)
        nc.sync.dma_start(out=cntb[:], in_=cnt[:])
        nc.gpsimd.collective_compute(
            "AllGather", OP.bypass, ins=[cntb[:].opt()], outs=[cntag[:].opt()],
            replica_groups=[list(range(NCORES))],
        )
        nc.gpsimd.collective_compute(
            "AllGather", OP.bypass, ins=[xn1b[:].opt()], outs=[agx[:].opt()],
            replica_groups=[list(range(NCORES))],
        )
        cnts = spool.tile([8, 8], F32, tag="cnts")
        nc.sync.dma_start(out=cnts[:].rearrange("c (e b) -> c e b", b=1), in_=cntag[:].rearrange("(c e) b -> c e b", e=8))
        ps_off = pb()
        nc.tensor.matmul(ps_off[0:8, 0:1], cnts[:], corelt[:], start=True, stop=True)
        off = spool.tile([8, 1], F32, tag="off")
        nc.vector.tensor_copy(off[:], ps_off[0:8, 0:1])
        nc.vector.tensor_scalar(incl[:], incl[:], off[:], None, op0=OP.add)
        ovf = spool.tile([8, TQ * 2], BF16, tag="vtmp", name="ovf")
        nc.vector.tensor_scalar(ovf[:], incl[:], float(CAP), None, op0=OP.is_ge)
        nc.vector.tensor_scalar(incl[:], incl[:], e2048[:], None, op0=OP.add)
        nc.vector.scalar_tensor_tensor(
            out=incl[:], in0=ovf[:], scalar=1e6, in1=incl[:], op0=OP.mult, op1=OP.add
        )
        nc.vector.tensor_tensor(out=incl[:], in0=incl[:], in1=S[:], op=OP.mult)
        for n in range(2):
            ps_t = pb()
            nc.tensor.matmul(
                ps_t[0:1, :], ones8[:], incl[:, n * 512 : (n + 1) * 512], start=True, stop=True
            )
            trh = spool.tile([1, 512], F32, tag="sq1r", name=f"trh{n}", bufs=2)
            nc.vector.tensor_copy(trh[:], ps_t[0:1, :])
            nc.sync.dma_start(
                out=ttmp[n * 512 : (n + 1) * 512, :].rearrange("a b -> b a"), in_=trh[:]
            )
        tgt_sb = spool.tile([P, 8], F32, tag="tgt_sb")
        nc.sync.dma_start(out=tgt_sb[:].rearrange("p (a b) -> p a b", b=1), in_=ttmp[:].rearrange("(a p) b -> p a b", p=P))
        w_ent = spool.tile([P, 8], F32, tag="w_ent")
        nc.sync.dma_start(out=w_ent[:].rearrange("p (a b) -> p a b", b=1), in_=wtmp[:].rearrange("(a p) b -> p a b", p=P))
        iT = spool.tile([P, 8], I32, tag="iT")
        nc.vector.tensor_copy(iT[:], tgt_sb[:])
        zz = spool.tile([P, 256], F32, tag="zz", bufs=1)
        nc.vector.memset(zz[:], 0.0)
        nc.sync.dma_start(out=table[:].rearrange("(p a) c -> p a c", p=P), in_=zz[:].rearrange("p (a c) -> p a c", c=2))
        for a in range(8):
            pay = spool.tile([P, 2], F32, tag="pay", bufs=2)
            nc.vector.tensor_copy(pay[:, 0:1], tokp1[:, a : a + 1])
            nc.vector.tensor_copy(pay[:, 1:2], w_ent[:, a : a + 1])
            nc.gpsimd.indirect_dma_start(
                out=table[:, :],
                out_offset=bass.IndirectOffsetOnAxis(ap=iT[:, a : a + 1], axis=0),
                in_=pay[:],
                in_offset=None,
                bounds_check=E * CAP - 1,
                oob_is_err=False,
            )
        nc.gpsimd.collective_compute(
            "ReduceScatter", OP.add, ins=[table[:].opt()], outs=[tabrs[:].opt()],
            replica_groups=[list(range(NCORES))],
        )

        # ================= A8: expert FFN =================
        fcT = load_w("fcT_in", H, BF16, "ws")
        projT = []
        for i in range(HT):
            t = wpool.tile([P, C], BF16, tag=f"wo{i}", name=f"projT{i}")
            nc.sync.dma_start(out=t[:], in_=io["projT_in"][i * P : (i + 1) * P, :])
            projT.append(t)

        for b4 in range(CAP // 512):
            tw, gidx = [], []
            bufT = [spool.tile([P, 512], BF16, tag=f"bufT{ct}", name=f"bufT{ct}_{b4}", bufs=1) for ct in range(CT)]
            for rsub in range(4):
                twt = spool.tile([P, 2], F32, tag="twt", bufs=4)
                nc.sync.dma_start(
                    out=twt[:], in_=tabrs[b4 * 512 + rsub * P : b4 * 512 + (rsub + 1) * P, :]
                )
                tw.append(twt)
                gif = spool.tile([P, 1], F32, tag="gif", bufs=4)
                nc.vector.tensor_scalar_add(gif[:], twt[:, 0:1], -1.0)
                gi = spool.tile([P, 1], I32, tag="gi", bufs=4)
                nc.vector.tensor_copy(gi[:], gif[:])
                gidx.append(gi)
                buf = spool.tile([P, C], BF16, tag="buf", bufs=1)
                nc.vector.memset(buf[:], 0.0)
                nc.gpsimd.indirect_dma_start(
                    out=buf[:], out_offset=None, in_=agx[:],
                    in_offset=bass.IndirectOffsetOnAxis(ap=gi[:, 0:1], axis=0),
                    bounds_check=NTOK - 1, oob_is_err=False,
                )
                for ct in range(CT):
                    ptb = pbb()
                    nc.tensor.transpose(ptb[:, 0:P], buf[:, ct * P : (ct + 1) * P], identb[:])
                    nc.vector.tensor_copy(bufT[ct][:, rsub * P : (rsub + 1) * P], ptb[:, 0:P])
            hT = []
            for hm in range(HT):
                ps_h = pb()
                for ct in range(CT):
                    nc.tensor.matmul(
                        ps_h[:], fcT[ct][:, hm * P : (hm + 1) * P], bufT[ct][:],
                        start=(ct == 0), stop=(ct == CT - 1),
                    )
                hR = spool.tile([P, 512], BF16, tag="hR", bufs=2)
                nc.scalar.activation(hR[:], ps_h[:], AF.Relu)
                ht = spool.tile([P, 512], BF16, tag=f"hT{hm}", bufs=1)
                nc.vector.tensor_tensor(out=ht[:], in0=hR[:], in1=hR[:], op=OP.mult)
                hT.append(ht)
            for rsub in range(4):
                ps_p = pa()
                for n in range(2):
                    for hm in range(HT):
                        nc.tensor.matmul(
                            ps_p[:, n * 512 : (n + 1) * 512],
                            hT[hm][:, rsub * P : (rsub + 1) * P],
                            projT[hm][:, n * 512 : (n + 1) * 512],
                            start=(hm == 0),
                            stop=(hm == HT - 1),
                        )
                bo = spool.tile([P, C], F32, tag="rp_raw", bufs=2)
                nc.vector.tensor_scalar_mul(bo[:], ps_p[:], tw[rsub][:, 1:2])
                nc.gpsimd.indirect_dma_start(
                    out=io["moe_out"][:, :],
                    out_offset=bass.IndirectOffsetOnAxis(ap=gidx[rsub][:, 0:1], axis=0),
                    in_=bo[:],
                    in_offset=None,
                    bounds_check=NTOK - 1,
                    oob_is_err=False,
                )


# ======================= host side =======================
_NC_CACHE = {}
TRACE = False
LAST_EXEC_NS = None
LAST_RESULTS = None


def _get_nc():
    if "nc" not in _NC_CACHE:
        _NC_CACHE["nc"] = _build()
    return _NC_CACHE["nc"]


def _prep_core_inputs(c, x, ve, cos, sin, fc_w, proj_w, shared):
    b, ch = c // 4, c % 4
    qs = ch * TQ
    ks0 = qs - 1024
    npad = max(0, -ks0)
    xc = np.zeros((TK, C), np.float32)
    xc[npad:] = x[b, max(ks0, 0) : qs + TQ]
    vec = np.zeros((TK, NKV * HD), np.float32)
    vec[npad:] = ve[b, max(ks0, 0) : qs + TQ]
    cosk = np.zeros((TK, 32), np.float32)
    sink = np.zeros((TK, 32), np.float32)
    cosk[npad:] = cos[0, max(ks0, 0) : qs + TQ, 0]
    sink[npad:] = sin[0, max(ks0, 0) : qs + TQ, 0]
    padb = np.zeros((TK, 1), np.float32)
    padb[:npad] = -30.0
    tokp1 = np.zeros((P, 8), np.float32)
    for a in range(8):
        j = a * P + np.arange(P)
        tokp1[:, a] = c * TQ + j // 2 + 1
    corelt = np.zeros((8, 1), np.float32)
    corelt[:c] = 1.0
    return dict(
        xT=np.ascontiguousarray(xc.T),
        xq=np.ascontiguousarray(x[b, qs : qs + TQ]),
        ve=vec, cosk=cosk, sink=sink, padb=padb,
        tokp1=tokp1, corelt=corelt,
        fcT=np.ascontiguousarray(fc_w[c].T.astype(ml_dtypes.bfloat16)),
        projT=np.ascontiguousarray(proj_w[c].T.astype(ml_dtypes.bfloat16)),
        **shared,
    )


def _make_in_maps(inputs):
    return _prep_all(**inputs)


def _prep_all(x, ve, cos, sin, c_q_w, c_k_w, c_v_w, c_proj_w, ve_gate_w,
              router_w, fc_w, proj_w, window_size):
    x = np.asarray(x, np.float32)
    ve = np.asarray(ve, np.float32)
    cos = np.asarray(cos, np.float32)
    sin = np.asarray(sin, np.float32)
    trimask = np.zeros((8 * P, TQ), ml_dtypes.bfloat16)
    kk = np.arange(P)[:, None]
    qq = np.arange(TQ)[None, :]
    for kt in range(4):
        trimask[kt * P : (kt + 1) * P] = (qq <= kk + P * kt).astype(ml_dtypes.bfloat16)
    for i in range(4):
        trimask[(4 + i) * P : (5 + i) * P] = (qq >= kk + P * i).astype(ml_dtypes.bfloat16)
    shared = dict(
        trimask=trimask,
        wqT=np.ascontiguousarray(np.asarray(c_q_w, np.float32).T),
        wkT=np.ascontiguousarray(np.asarray(c_k_w, np.float32).T),
        wvT=np.ascontiguousarray(np.asarray(c_v_w, np.float32).T),
        gateT=np.ascontiguousarray(np.asarray(ve_gate_w, np.float32).T),
        woT=np.ascontiguousarray(np.asarray(c_proj_w, np.float32).T.astype(ml_dtypes.bfloat16)),
        routerT=np.ascontiguousarray(np.asarray(router_w, np.float32).T),
        e2048=(np.arange(8, dtype=np.float32) * CAP).reshape(8, 1),
        iota8=np.broadcast_to(np.arange(8, dtype=np.float32), (P, 8)).copy(),
    )
    fc_w = np.asarray(fc_w, np.float32)
    proj_w = np.asarray(proj_w, np.float32)
    return [
        _prep_core_inputs(c, x, ve, cos, sin, fc_w, proj_w, shared)
        for c in range(NCORES)
    ]


def kernel(**inputs):
    in_maps = _prep_all(**inputs)
    nc = _get_nc()
    global LAST_EXEC_NS, LAST_RESULTS
    res = run_bass_kernel_spmd(nc, in_maps, core_ids=list(range(NCORES)), trace=TRACE)
    LAST_EXEC_NS = res.exec_time_ns
    LAST_RESULTS = res
    out = np.empty((NTOK, C), np.float32)
    rw = np.empty((NTOK, E), np.float32)
    for c in range(NCORES):
        out[c * TQ : (c + 1) * TQ] = res.results[c]["x1o"]
        rw[c * TQ : (c + 1) * TQ] = res.results[c]["rwo"]
    for c in range(NCORES):
        out += res.results[c]["moeo"]
    return out.reshape(B, T, C), rw.reshape(B, T, E)


# revision 22
# speedup vs baseline: 137.6612x; 137.6612x over previous
"""Trainium2 Bass kernel for nn_BlockMoE (attention + top-2 MoE block), 8-core SPMD.

Sharding: attention is T-sharded (core c handles batch c//4, a 512-token chunk
with a 1024-token KV halo); MoE is expert-parallel (core e owns expert e).
Cross-core exchange: AllGather of per-core expert counts, AllGather of xn1
(bf16), ReduceScatter of the dispatch table.  Host only shards inputs /
gathers and sums outputs.
"""
import sys

for _p in ("/opt/trn_rl_repo",):
    if _p not in sys.path:
        sys.path.insert(0, _p)

import numpy as np
import ml_dtypes

import concourse.bass as bass
import concourse.mybir as mybir
import bass_rust as _bass_rust
from concourse.tile import TileContext
from concourse.masks import make_identity
from concourse.bass_utils import run_bass_kernel_spmd

F32 = mybir.dt.float32
F32R = mybir.dt.float32r
BF16 = mybir.dt.bfloat16
I32 = mybir.dt.int32
U32 = mybir.dt.uint32
AF = mybir.ActivationFunctionType
OP = mybir.AluOpType
AX = mybir.AxisListType

P = 128
B, T, C = 2, 2048, 1024
NH, NKV, HD = 16, 8, 64
E, H = 8, 2048
NTOK = B * T
TQ = 512
TK = 1536
KT = TK // P       # 12
QT = TQ // P       # 4
CT = C // P        # 8
HT = H // P        # 16
CAP = 2 * NTOK * 2 // E   # 2048
EPS = float(np.finfo(np.float32).eps)
NCORES = 8
QK_DT = F32R       # dtype for q^T/k^T storage + scores matmul


def _split_waits(nc, max_waits=1):
    """This walrus accepts at most 1 sem-wait per instruction; hoist the rest
    onto nops inserted just before."""
    n_fixed = 0
    for bb in nc.main_func.blocks:
        targets = [
            ins
            for ins in bb.instructions
            if ins.sync_info is not None
            and ins.sync_info.on_wait
            and len(ins.sync_info.on_wait) > max_waits
            and ins.engine != mybir.EngineType.Unassigned
        ]
        for ins in targets:
            waits = list(ins.sync_info.on_wait)
            keep, rest = waits[:max_waits], waits[max_waits:]
            nops = []
            for i in range(0, len(rest), max_waits):
                chunk = rest[i : i + max_waits]
                bi = nc.engines[ins.engine].nop(nofuse=True)
                nop_inst = bi.ins
                for bb2 in nc.main_func.blocks:
                    if nop_inst in bb2.instructions:
                        bb2.instructions.remove(nop_inst)
                nop_inst.sync_info = _bass_rust.SyncInfo(on_wait=chunk, on_update=[])
                nops.append(nop_inst)
                n_fixed += 1
            ins.sync_info = _bass_rust.SyncInfo(
                on_wait=keep, on_update=list(ins.sync_info.on_update or [])
            )
            pos = bb.instructions.index(ins)
            bb.instructions[pos:pos] = nops
    return n_fixed


def _build():
    nc = bass.Bass("TRN2", target_bir_lowering=False)
    dp = nc.declare_dram_parameter
    io = {}
    io["xT_in"] = dp("xT", [C, TK], F32R, isOutput=False)
    io["xq_in"] = dp("xq", [TQ, C], F32, isOutput=False)
    io["ve_in"] = dp("ve", [TK, NKV * HD], F32, isOutput=False)
    io["cosk_in"] = dp("cosk", [TK, 32], F32, isOutput=False)
    io["sink_in"] = dp("sink", [TK, 32], F32, isOutput=False)
    io["padb_in"] = dp("padb", [TK, 1], F32, isOutput=False)
    io["trimask_in"] = dp("trimask", [8 * P, TQ], BF16, isOutput=False)
    io["wqT_in"] = dp("wqT", [C, NH * HD], F32R, isOutput=False)
    io["wkT_in"] = dp("wkT", [C, NKV * HD], F32R, isOutput=False)
    io["wvT_in"] = dp("wvT", [C, NKV * HD], F32R, isOutput=False)
    io["gateT_in"] = dp("gateT", [32, 8], F32R, isOutput=False)
    io["woT_in"] = dp("woT", [C, C], BF16, isOutput=False)
    io["routerT_in"] = dp("routerT", [C, E], F32, isOutput=False)
    io["fcT_in"] = dp("fcT", [C, H], BF16, isOutput=False)
    io["projT_in"] = dp("projT", [H, C], BF16, isOutput=False)
    io["corelt_in"] = dp("corelt", [8, 1], F32, isOutput=False)
    io["e2048_in"] = dp("e2048", [8, 1], F32, isOutput=False)
    io["iota8_in"] = dp("iota8", [P, 8], F32, isOutput=False)
    io["tokp1_in"] = dp("tokp1", [P, 8], F32, isOutput=False)
    io["x1_out"] = dp("x1o", [TQ, C], F32, isOutput=True)
    io["rw_out"] = dp("rwo", [TQ, E], F32, isOutput=True)
    io["moe_out"] = dp("moeo", [NTOK, C], F32, isOutput=True)

    with TileContext(nc) as tc:
        _program(nc, tc, io)
    _split_waits(nc)
    return nc


def _program(nc, tc, io):
    import contextlib

    ctx = contextlib.ExitStack()
    with ctx:
        const = ctx.enter_context(tc.tile_pool(name="const", bufs=1))
        wpool = ctx.enter_context(tc.tile_pool(name="wpool", bufs=1))
        spool = ctx.enter_context(tc.tile_pool(name="spool", bufs=2))
        hold = ctx.enter_context(tc.tile_pool(name="hold", bufs=1))
        psA = ctx.enter_context(tc.tile_pool(name="psA", bufs=2, space="PSUM"))
        psB = ctx.enter_context(tc.tile_pool(name="psB", bufs=3, space="PSUM"))
        psY = ctx.enter_context(tc.tile_pool(name="psY", bufs=1, space="PSUM"))

        _ctr = [0]

        def pa():
            _ctr[0] += 1
            return psA.tile([P, 1024], F32, tag="A", name=f"psA_{_ctr[0]}")

        def pb(shape=None):
            _ctr[0] += 1
            return psB.tile([P, 512], F32, tag="B", name=f"psB_{_ctr[0]}")

        def pbb():
            _ctr[0] += 1
            return psB.tile([P, 512], BF16, tag="B", name=f"psBb_{_ctr[0]}")

        dram = ctx.enter_context(tc.tile_pool(name="dram", bufs=1, space="DRAM"))

        # ===== constants =====
        ident = const.tile([P, P], F32)
        make_identity(nc, ident[:])
        identb = const.tile([P, P], BF16)
        nc.vector.tensor_copy(identb[:], ident[:])
        ones1b = const.tile([P, 1], BF16)
        nc.vector.memset(ones1b[:], 1.0)
        ones8 = const.tile([8, 1], F32)
        nc.vector.memset(ones8[:], 1.0)
        onesb = const.tile([P, HD], BF16)
        nc.vector.memset(onesb[:], 1.0)
        padb = const.tile([P, KT], F32)
        nc.sync.dma_start(out=padb[:].rearrange("p (a b) -> p a b", b=1), in_=io["padb_in"][:].rearrange("(a p) b -> p a b", p=P))
        iota8 = const.tile([P, 8], F32)
        nc.sync.dma_start(out=iota8[:], in_=io["iota8_in"][:])
        tokp1 = const.tile([P, 8], F32)
        nc.sync.dma_start(out=tokp1[:], in_=io["tokp1_in"][:])
        corelt = const.tile([8, 1], F32)
        nc.sync.dma_start(out=corelt[:], in_=io["corelt_in"][:])
        epsc = const.tile([P, 1], F32)
        nc.vector.memset(epsc[:], EPS)
        e2048 = const.tile([8, 1], F32)
        nc.sync.dma_start(out=e2048[:], in_=io["e2048_in"][:])
        gateT = const.tile([32, 8], F32R)
        nc.sync.dma_start(out=gateT[:], in_=io["gateT_in"][:])
        routerT = const.tile([P, CT * E], F32)
        nc.sync.dma_start(
            out=routerT[:].rearrange("p (a e) -> p a e", e=E), in_=io["routerT_in"][:].rearrange("(a p) e -> p a e", p=P)
        )
        masks = []
        for i in range(8):
            mt = const.tile([P, TQ], BF16, tag=f"mask{i}")
            nc.sync.dma_start(out=mt[:], in_=io["trimask_in"][i * P : (i + 1) * P, :])
            masks.append(mt)
        cosk, sink = [], []
        for tt in range(KT):
            ctile = const.tile([P, 32], F32, tag=f"cos{tt}")
            stile = const.tile([P, 32], F32, tag=f"sin{tt}")
            nc.sync.dma_start(out=ctile[:], in_=io["cosk_in"][tt * P : (tt + 1) * P, :])
            nc.sync.dma_start(out=stile[:], in_=io["sink_in"][tt * P : (tt + 1) * P, :])
            cosk.append(ctile)
            sink.append(stile)

        # resident attention out (transposed q/k spilled to DRAM)
        yT = hold.tile([HD, NH * TQ], BF16)
        rcol = hold.tile([P, KT], F32)
        S = hold.tile([8, TQ * 2], BF16)

        # DRAM scratch
        kT_d = dram.tile([HD, NKV * TK], QK_DT)
        vext_d = dram.tile([NKV * TK, HD + 1], BF16)
        qT_d = dram.tile([HD, NH * TQ], QK_DT)
        xn1b = dram.tile([TQ, C], BF16)
        agx = dram.tile([NTOK, C], BF16, addr_space="Shared")
        cntb = dram.tile([8, 1], F32)
        cntag = dram.tile([NCORES * 8, 1], F32, addr_space="Shared")
        wtmp = dram.tile([TQ * 2, 1], F32)
        ttmp = dram.tile([TQ * 2, 1], F32)
        table = dram.tile([E * CAP, 2], F32)
        tabrs = dram.tile([CAP, 2], F32)

        # weight slots (reused across passes)
        def load_w(name, cols, dtype, tagp):
            tiles = []
            for i in range(CT):
                t = wpool.tile([P, cols], dtype, tag=f"{tagp}{i}")
                nc.sync.dma_start(out=t[:], in_=io[name][i * P : (i + 1) * P, :])
                tiles.append(t)
            return tiles

        def stream_x(tt, tag="xs"):
            xs = []
            for ct in range(CT):
                t = spool.tile([P, P], F32R, tag=f"{tag}{ct}", bufs=2)
                nc.sync.dma_start(
                    out=t[:], in_=io["xT_in"][ct * P : (ct + 1) * P, tt * P : (tt + 1) * P]
                )
                xs.append(t)
            return xs

        def rope_norm(nc, ps_raw, nh, tt, dst, dst_col):
            """psum raw [P, nh*HD] -> rope -> per-head rms -> transpose into
            dst[:, head*W + dst_col*P : ...] (dst width W per head)."""
            rc = rcol[:, tt : tt + 1]
            raw = spool.tile([P, 16, HD], F32, tag="rp_raw")
            r3 = raw[:, 0:nh, :]
            nc.vector.tensor_scalar_mul(r3, ps_raw[:].rearrange("p (h d) -> p h d", h=nh), rc)
            cb = cosk[tt][:].rearrange("p (h d) -> p h d", h=1).to_broadcast([P, nh, 32])
            sb = sink[tt][:].rearrange("p (h d) -> p h d", h=1).to_broadcast([P, nh, 32])
            m1 = spool.tile([P, 16, 32], F32, tag="rp_m1", bufs=1)
            m2 = spool.tile([P, 16, 32], F32, tag="rp_m2", bufs=1)
            hat = spool.tile([P, 16, HD], F32, tag="rp_hn", bufs=1)
            h3 = hat[:, 0:nh, :]
            nc.vector.tensor_tensor(out=m1[:, 0:nh], in0=r3[:, :, 0:32], in1=cb, op=OP.mult)
            nc.vector.tensor_tensor(out=m2[:, 0:nh], in0=r3[:, :, 32:64], in1=sb, op=OP.mult)
            nc.vector.tensor_tensor(out=h3[:, :, 0:32], in0=m1[:, 0:nh], in1=m2[:, 0:nh], op=OP.add)
            nc.vector.tensor_tensor(out=m1[:, 0:nh], in0=r3[:, :, 32:64], in1=cb, op=OP.mult)
            nc.vector.tensor_tensor(out=m2[:, 0:nh], in0=r3[:, :, 0:32], in1=sb, op=OP.mult)
            nc.vector.tensor_tensor(out=h3[:, :, 32:64], in0=m1[:, 0:nh], in1=m2[:, 0:nh], op=OP.subtract)
            sq = spool.tile([P, 16, HD], F32, tag="rp_raw", name="rp_sqv")
            nc.vector.tensor_tensor(out=sq[:, 0:nh], in0=h3, in1=h3, op=OP.mult)
            ssum = spool.tile([P, 16], F32, tag="rp_ss")
            nc.vector.tensor_reduce(ssum[:, 0:nh], sq[:, 0:nh], axis=AX.X, op=OP.add)
            rh = spool.tile([P, 16], F32, tag="rp_rh")
            nc.scalar.activation(rh[:, 0:nh], ssum[:, 0:nh], AF.Sqrt, bias=epsc[:], scale=1.0 / HD)
            nc.vector.reciprocal(rh[:, 0:nh], rh[:, 0:nh])
            nc.vector.tensor_tensor(
                out=h3, in0=h3, in1=rh[:, 0:nh].to_broadcast([P, nh, HD]), op=OP.mult
            )
            asm = spool.tile([HD, 16 * P], QK_DT, tag="rp_asm", bufs=1)
            for h in range(nh):
                pt = pb()
                nc.tensor.transpose(pt[0:HD, 0:P], hat[:, h, :], ident[:])
                nc.vector.tensor_copy(asm[:, h * P : (h + 1) * P], pt[0:HD, 0:P])
            nc.sync.dma_start(
                out=dst[:].rearrange("a (h w) -> a h w", h=nh)[:, :, dst_col * P : (dst_col + 1) * P],
                in_=asm[:, 0 : nh * P].rearrange("a (h w) -> a h w", h=nh),
            )

        # ================= pass K (+ rms1 fused) =================
        wk = load_w("wkT_in", NKV * HD, F32R, "ws")
        for tt in range(KT):
            xs = stream_x(tt)
            # rms1 for this token tile
            ss = pb()
            for ct in range(CT):
                sq = spool.tile([P, P], BF16, tag="sq1", bufs=2)
                nc.scalar.activation(sq[:], xs[ct][:].bitcast(F32), AF.Square)
                nc.tensor.matmul(ss[0:1, 0:P], ones1b[:], sq[:], start=(ct == 0), stop=(ct == CT - 1))
            sq1 = spool.tile([1, P], F32, tag="sq1r", bufs=2)
            nc.vector.tensor_copy(sq1[:], ss[0:1, 0:P])
            pt = pb()
            nc.tensor.transpose(pt[0:P, 0:1], sq1[:], ident[0:1, 0:1])
            nc.scalar.activation(rcol[:, tt : tt + 1], pt[0:P, 0:1], AF.Sqrt, bias=epsc[:], scale=1.0 / C)
            nc.vector.reciprocal(rcol[:, tt : tt + 1], rcol[:, tt : tt + 1])
            # k
            ps_k = pb()
            for ct in range(CT):
                nc.tensor.matmul(ps_k[:], xs[ct][:], wk[ct][:], start=(ct == 0), stop=(ct == CT - 1))
            rope_norm(nc, ps_k, NKV, tt, kT_d, tt)

        # ================= pass V (+ gate) =================
        wv = load_w("wvT_in", NKV * HD, F32R, "ws")
        for tt in range(KT):
            xs = stream_x(tt)
            rc = rcol[:, tt : tt + 1]
            ps_v = pb()
            for ct in range(CT):
                nc.tensor.matmul(ps_v[:], xs[ct][:], wv[ct][:], start=(ct == 0), stop=(ct == CT - 1))
            ps_g = pb()
            nc.tensor.matmul(ps_g[0:P, 0:8], xs[0][0:32, :], gateT[:], start=True, stop=True)
            ga = spool.tile([P, NKV], F32, tag="ga")
            nc.scalar.activation(ga[:], ps_g[0:P, 0:8], AF.Sigmoid, scale=rc)
            vtmp = spool.tile([P, NKV * HD], F32, tag="vtmp")
            nc.vector.tensor_scalar_mul(vtmp[:], ps_v[:], rc)
            vet = spool.tile([P, NKV * HD], F32, tag="vet", bufs=2)
            nc.sync.dma_start(out=vet[:], in_=io["ve_in"][tt * P : (tt + 1) * P, :])
            nc.vector.tensor_tensor(
                out=vet[:].rearrange("p (h d) -> p h d", h=NKV),
                in0=vet[:].rearrange("p (h d) -> p h d", h=NKV),
                in1=ga[:].to_broadcast([P, NKV, HD]),
                op=OP.mult,
            )
            nc.vector.scalar_tensor_tensor(
                out=vtmp[:], in0=vet[:], scalar=2.0, in1=vtmp[:], op0=OP.mult, op1=OP.add
            )
            vtile = spool.tile([P, NKV, HD + 1], BF16, tag="vtile", bufs=2)
            nc.vector.tensor_copy(vtile[:, :, 0:HD], vtmp[:].rearrange("p (h d) -> p h d", h=NKV))
            nc.vector.memset(vtile[:, :, HD : HD + 1], 1.0)
            nc.sync.dma_start(
                out=vext_d[:].rearrange("(h r) d -> h r d", h=NKV)[
                    :, tt * P : (tt + 1) * P, :
                ].rearrange("h p d -> p h d"),
                in_=vtile[:],
            )

        # ================= pass Q =================
        wq = load_w("wqT_in", NH * HD, F32R, "ws")
        for tt in range(8, KT):
            xs = stream_x(tt)
            ps_q = pa()
            for half in range(2):
                for ct in range(CT):
                    nc.tensor.matmul(
                        ps_q[:, half * 512 : (half + 1) * 512],
                        xs[ct][:],
                        wq[ct][:, half * 512 : (half + 1) * 512],
                        start=(ct == 0),
                        stop=(ct == CT - 1),
                    )
            rope_norm(nc, ps_q, NH, tt, qT_d, tt - 8)

        # ================= A4: scores/softmax/pv =================
        yTv = yT[:].rearrange("a (h w) -> a h w", h=NH)
        for h in range(NKV):
            vexth = spool.tile([P, KT, HD + 1], BF16, tag="vexth", bufs=2)
            nc.sync.dma_start(
                out=vexth[:],
                in_=vext_d[:].rearrange("(h r) d -> h r d", h=NKV)[h, :, :].rearrange(
                    "(t p) d -> p t d", p=P
                ),
            )
            kTh = spool.tile([HD, TK], QK_DT, tag="kTh", bufs=1)
            nc.sync.dma_start(out=kTh[:], in_=kT_d[:].rearrange("a (h w) -> a h w", h=NKV)[:, h, :])
            for qh in (2 * h, 2 * h + 1):
                qTh = spool.tile([HD, TQ], QK_DT, tag="qTh", bufs=1)
                nc.sync.dma_start(out=qTh[:], in_=qT_d[:].rearrange("a (h w) -> a h w", h=NH)[:, qh, :])
                ps_y = psY.tile([HD + 1, TQ], F32, tag="Y")
                for kt in range(KT):
                    ps_s = pb()
                    nc.tensor.matmul(
                        ps_s[:], kTh[:, kt * P : (kt + 1) * P], qTh[:],
                        start=True, stop=True,
                    )
                    pT = spool.tile([P, TQ], BF16, tag="pT", bufs=2)
                    nc.scalar.activation(
                        pT[:], ps_s[:], AF.Exp, bias=padb[:, kt : kt + 1], scale=0.125
                    )
                    if kt < 4:
                        nc.vector.tensor_tensor(out=pT[:], in0=pT[:], in1=masks[kt][:], op=OP.mult)
                    elif kt >= 8:
                        nc.vector.tensor_tensor(out=pT[:], in0=pT[:], in1=masks[kt - 4][:], op=OP.mult)
                    nc.tensor.matmul(
                        ps_y[:], vexth[:, kt, :], pT[:], start=(kt == 0), stop=(kt == KT - 1)
                    )
                zrow = spool.tile([P, TQ], F32, tag="vtmp")
                nc.vector.reciprocal(zrow[HD : HD + 1, :], ps_y[HD : HD + 1, :])
                bcs = spool.tile([P, TQ], BF16, tag="bcs")
                nc.vector.tensor_copy(bcs[HD : HD + 1, :], zrow[HD : HD + 1, :])
                ps_b = pb()
                nc.tensor.matmul(
                    ps_b[0:HD, :], onesb[HD : HD + 1, :], bcs[HD : HD + 1, :],
                    start=True, stop=True,
                )
                nc.vector.tensor_copy(bcs[0:HD, :], ps_b[0:HD, :])
                nc.vector.tensor_tensor(
                    out=yTv[:, qh, :], in0=ps_y[0:HD, :], in1=bcs[0:HD, :], op=OP.mult
                )

        # ================= A5/A6: proj+residual+rms2+router =================
        woT = []
        for i in range(NH):
            wt = wpool.tile([HD, C], BF16, tag=f"wo{i}", name=f"woT{i}")
            nc.sync.dma_start(out=wt[:], in_=io["woT_in"][i * HD : (i + 1) * HD, :])
            woT.append(wt)
        for m in range(QT):
            ps_o = pa()
            for n in range(2):
                for qh in range(NH):
                    nc.tensor.matmul(
                        ps_o[:, n * 512 : (n + 1) * 512],
                        yTv[:, qh, m * P : (m + 1) * P],
                        woT[qh][:, n * 512 : (n + 1) * 512],
                        start=(qh == 0),
                        stop=(qh == NH - 1),
                    )
            xqt = spool.tile([P, C], F32, tag="rp_raw", bufs=2)
            nc.sync.dma_start(out=xqt[:], in_=io["xq_in"][m * P : (m + 1) * P, :])
            x1m = spool.tile([P, C], F32, tag="x1m", bufs=1)
            nc.vector.tensor_tensor(out=x1m[:], in0=ps_o[:], in1=xqt[:], op=OP.add)
            nc.sync.dma_start(out=io["x1_out"][m * P : (m + 1) * P, :], in_=x1m[:])
            sq = spool.tile([P, C], F32, tag="rp_hn", bufs=1)
            nc.vector.tensor_tensor(out=sq[:], in0=x1m[:], in1=x1m[:], op=OP.mult)
            ssum = spool.tile([P, 1], F32, tag="ss_a6")
            nc.vector.tensor_reduce(ssum[:], sq[:], axis=AX.X, op=OP.add)
            r1 = spool.tile([P, 1], F32, tag="r1_a6")
            nc.scalar.activation(r1[:], ssum[:], AF.Sqrt, bias=epsc[:], scale=1.0 / C)
            nc.vector.reciprocal(r1[:], r1[:])
            xn1 = spool.tile([P, C], F32, tag="rp_sq2", bufs=2)
            nc.vector.tensor_scalar_mul(xn1[:], x1m[:], r1[:])
            xn1c = spool.tile([P, C], BF16, tag="xn1c", bufs=1)
            nc.vector.tensor_copy(xn1c[:], xn1[:])
            nc.sync.dma_start(out=xn1b[m * P : (m + 1) * P, :], in_=xn1c[:])
            # router (plain f32 matmuls, exact)
            ps_r = pb()
            for ct in range(CT):
                ptr = pb()
                nc.tensor.transpose(ptr[:, 0:P], xn1[:, ct * P : (ct + 1) * P], ident[:])
                xn1T = spool.tile([P, P], F32, tag="xn1T", bufs=2)
                nc.vector.tensor_copy(xn1T[:], ptr[:, 0:P])
                nc.tensor.matmul(
                    ps_r[0:P, 0:E], xn1T[:], routerT[:, ct * E : (ct + 1) * E],
                    start=(ct == 0), stop=(ct == CT - 1),
                )
            er = spool.tile([P, E], F32, tag="er")
            nc.scalar.activation(er[:], ps_r[0:P, 0:E], AF.Exp)
            es = spool.tile([P, 1], F32, tag="es")
            nc.vector.tensor_reduce(es[:], er[:], axis=AX.X, op=OP.add)
            nc.vector.reciprocal(es[:], es[:])
            rw = spool.tile([P, E], F32, tag="rw")
            nc.vector.tensor_scalar_mul(rw[:], er[:], es[:])
            nc.sync.dma_start(out=io["rw_out"][m * P : (m + 1) * P, :], in_=rw[:])
            tv = spool.tile([P, 8], F32, tag="tv")
            ti = spool.tile([P, 8], U32, tag="ti")
            nc.vector.max_with_indices(tv[:], ti[:], rw[:])
            tif = spool.tile([P, 8], F32, tag="tif")
            nc.vector.tensor_copy(tif[:], ti[:])
            den = spool.tile([P, 1], F32, tag="den")
            nc.vector.tensor_tensor(out=den[:], in0=tv[:, 0:1], in1=tv[:, 1:2], op=OP.add)
            nc.vector.tensor_scalar_add(den[:], den[:], 1e-10)
            nc.vector.reciprocal(den[:], den[:])
            swp = spool.tile([P, 2], F32, tag="swp")
            nc.vector.tensor_scalar_mul(swp[:], tv[:, 0:2], den[:])
            nc.sync.dma_start(
                out=wtmp[m * 256 : (m + 1) * 256, :].rearrange("(p c) b -> p c b", c=2),
                in_=swp[:].rearrange("p (c b) -> p c b", b=1),
            )
            for k in range(2):
                oh = spool.tile([P, 8], F32, tag="oh")
                nc.vector.tensor_tensor(
                    out=oh[:], in0=tif[:, k : k + 1].to_broadcast([P, 8]), in1=iota8[:],
                    op=OP.is_equal,
                )
                pso = pb()
                nc.tensor.transpose(pso[0:8, 0:P], oh[:], ident[:])
                nc.vector.tensor_copy(
                    S[:].rearrange("e (x c) -> e x c", c=2)[:, m * P : (m + 1) * P, k],
                    pso[0:8, 0:P],
                )

        # ================= A7: routing =================
        zc8 = const.tile([8, 1], F32)
        nc.vector.memset(zc8[:], 0.0)
        incl = hold.tile([8, TQ * 2], F32)
        nc.vector.tensor_tensor_scan(
            incl[:], S[:], zc8[:].to_broadcast([8, TQ * 2]), 0.0, op0=OP.add, op1=OP.add
        )
        cnt = spool.tile([8, 1], F32, tag="cnt")
        nc.vector.tensor_copy(cnt[:], incl[:, TQ * 2 - 1 : TQ * 2])
        nc.vector.tensor_tensor(out=incl[:], in0=incl[:], in1=S[:], op=OP.subtract)
        nc.sync.dma_start(out=cntb[:], in_=cnt[:])
        nc.gpsimd.collective_compute(
            "AllGather", OP.bypass, ins=[cntb[:].opt()], outs=[cntag[:].opt()],
            replica_groups=[list(range(NCORES))],
        )
        nc.gpsimd.collective_compute(
            "AllGather", OP.bypass, ins=[xn1b[:].opt()], outs=[agx[:].opt()],
            replica_groups=[list(range(NCORES))],
        )
        cnts = spool.tile([8, 8], F32, tag="cnts")
        nc.sync.dma_start(out=cnts[:].rearrange("c (e b) -> c e b", b=1), in_=cntag[:].rearrange("(c e) b -> c e b", e=8))
        ps_off = pb()
        nc.tensor.matmul(ps_off[0:8, 0:1], cnts[:], corelt[:], start=True, stop=True)
        off = spool.tile([8, 1], F32, tag="off")
        nc.vector.tensor_copy(off[:], ps_off[0:8, 0:1])
        nc.vector.tensor_scalar(incl[:], incl[:], off[:], None, op0=OP.add)
        ovf = spool.tile([8, TQ * 2], BF16, tag="vtmp", name="ovf")
        nc.vector.tensor_scalar(ovf[:], incl[:], float(CAP), None, op0=OP.is_ge)
        nc.vector.tensor_scalar(incl[:], incl[:], e2048[:], None, op0=OP.add)
        nc.vector.scalar_tensor_tensor(
            out=incl[:], in0=ovf[:], scalar=1e6, in1=incl[:], op0=OP.mult, op1=OP.add
        )
        nc.vector.tensor_tensor(out=incl[:], in0=incl[:], in1=S[:], op=OP.mult)
        for n in range(2):
            ps_t = pb()
            nc.tensor.matmul(
                ps_t[0:1, :], ones8[:], incl[:, n * 512 : (n + 1) * 512], start=True, stop=True
            )
            trh = spool.tile([1, 512], F32, tag="sq1r", name=f"trh{n}", bufs=2)
            nc.vector.tensor_copy(trh[:], ps_t[0:1, :])
            nc.sync.dma_start(
                out=ttmp[n * 512 : (n + 1) * 512, :].rearrange("a b -> b a"), in_=trh[:]
            )
        tgt_sb = spool.tile([P, 8], F32, tag="tgt_sb")
        nc.sync.dma_start(out=tgt_sb[:].rearrange("p (a b) -> p a b", b=1), in_=ttmp[:].rearrange("(a p) b -> p a b", p=P))
        w_ent = spool.tile([P, 8], F32, tag="w_ent")
        nc.sync.dma_start(out=w_ent[:].rearrange("p (a b) -> p a b", b=1), in_=wtmp[:].rearrange("(a p) b -> p a b", p=P))
        iT = spool.tile([P, 8], I32, tag="iT")
        nc.vector.tensor_copy(iT[:], tgt_sb[:])
        zz = spool.tile([P, 256], F32, tag="zz", bufs=1)
        nc.vector.memset(zz[:], 0.0)
        nc.sync.dma_start(out=table[:].rearrange("(p a) c -> p a c", p=P), in_=zz[:].rearrange("p (a c) -> p a c", c=2))
        for a in range(8):
            pay = spool.tile([P, 2], F32, tag="pay", bufs=2)
            nc.vector.tensor_copy(pay[:, 0:1], tokp1[:, a : a + 1])
            nc.vector.tensor_copy(pay[:, 1:2], w_ent[:, a : a + 1])
            nc.gpsimd.indirect_dma_start(
                out=table[:, :],
                out_offset=bass.IndirectOffsetOnAxis(ap=iT[:, a : a + 1], axis=0),
                in_=pay[:],
                in_offset=None,
                bounds_check=E * CAP - 1,
                oob_is_err=False,
            )
        nc.gpsimd.collective_compute(
            "ReduceScatter", OP.add, ins=[table[:].opt()], outs=[tabrs[:].opt()],
            replica_groups=[list(range(NCORES))],
        )

        # ================= A8: expert FFN =================
        fcT = load_w("fcT_in", H, BF16, "ws")
        projT = []
        for i in range(HT):
            t = wpool.tile([P, C], BF16, tag=f"wo{i}", name=f"projT{i}")
            nc.sync.dma_start(out=t[:], in_=io["projT_in"][i * P : (i + 1) * P, :])
            projT.append(t)

        for b4 in range(CAP // 512):
            tw, gidx = [], []
            bufT = [spool.tile([P, 512], BF16, tag=f"bufT{ct}", name=f"bufT{ct}_{b4}", bufs=1) for ct in range(CT)]
            for rsub in range(4):
                twt = spool.tile([P, 2], F32, tag="twt", bufs=4)
                nc.sync.dma_start(
                    out=twt[:], in_=tabrs[b4 * 512 + rsub * P : b4 * 512 + (rsub + 1) * P, :]
                )
                tw.append(twt)
                gif = spool.tile([P, 1], F32, tag="gif", bufs=4)
                nc.vector.tensor_scalar_add(gif[:], twt[:, 0:1], -1.0)
                gi = spool.tile([P, 1], I32, tag="gi", bufs=4)
                nc.vector.tensor_copy(gi[:], gif[:])
                gidx.append(gi)
                buf = spool.tile([P, C], BF16, tag="buf", bufs=1)
                nc.vector.memset(buf[:], 0.0)
                nc.gpsimd.indirect_dma_start(
                    out=buf[:], out_offset=None, in_=agx[:],
                    in_offset=bass.IndirectOffsetOnAxis(ap=gi[:, 0:1], axis=0),
                    bounds_check=NTOK - 1, oob_is_err=False,
                )
                for ct in range(CT):
                    ptb = pbb()
                    nc.tensor.transpose(ptb[:, 0:P], buf[:, ct * P : (ct + 1) * P], identb[:])
                    nc.vector.tensor_copy(bufT[ct][:, rsub * P : (rsub + 1) * P], ptb[:, 0:P])
            hT = []
            for hm in range(HT):
                ps_h = pb()
                for ct in range(CT):
                    nc.tensor.matmul(
                        ps_h[:], fcT[ct][:, hm * P : (hm + 1) * P], bufT[ct][:],
                        start=(ct == 0), stop=(ct == CT - 1),
                    )
                hR = spool.tile([P, 512], BF16, tag="hR", bufs=2)
                nc.scalar.activation(hR[:], ps_h[:], AF.Relu)
                ht = spool.tile([P, 512], BF16, tag=f"hT{hm}", bufs=1)
                nc.vector.tensor_tensor(out=ht[:], in0=hR[:], in1=hR[:], op=OP.mult)
                hT.append(ht)
            for rsub in range(4):
                ps_p = pa()
                for n in range(2):
                    for hm in range(HT):
                        nc.tensor.matmul(
                            ps_p[:, n * 512 : (n + 1) * 512],
                            hT[hm][:, rsub * P : (rsub + 1) * P],
                            projT[hm][:, n * 512 : (n + 1) * 512],
                            start=(hm == 0),
                            stop=(hm == HT - 1),
                        )
                bo = spool.tile([P, C], F32, tag="rp_raw", bufs=2)
                nc.vector.tensor_scalar_mul(bo[:], ps_p[:], tw[rsub][:, 1:2])
                nc.gpsimd.indirect_dma_start(
                    out=io["moe_out"][:, :],
                    out_offset=bass.IndirectOffsetOnAxis(ap=gidx[rsub][:, 0:1], axis=0),
                    in_=bo[:],
                    in_offset=None,
                    bounds_check=NTOK - 1,
                    oob_is_err=False,
                )


# ======================= host side =======================
_NC_CACHE = {}
TRACE = False
LAST_EXEC_NS = None
LAST_RESULTS = None


def _get_nc():
    if "nc" not in _NC_CACHE:
        _NC_CACHE["nc"] = _build()
    return _NC_CACHE["nc"]


def _prep_core_inputs(c, x, ve, cos, sin, fc_w, proj_w, shared):
    b, ch = c // 4, c % 4
    qs = ch * TQ
    ks0 = qs - 1024
    npad = max(0, -ks0)
    xc = np.zeros((TK, C), np.float32)
    xc[npad:] = x[b, max(ks0, 0) : qs + TQ]
    vec = np.zeros((TK, NKV * HD), np.float32)
    vec[npad:] = ve[b, max(ks0, 0) : qs + TQ]
    cosk = np.zeros((TK, 32), np.float32)
    sink = np.zeros((TK, 32), np.float32)
    cosk[npad:] = cos[0, max(ks0, 0) : qs + TQ, 0]
    sink[npad:] = sin[0, max(ks0, 0) : qs + TQ, 0]
    padb = np.zeros((TK, 1), np.float32)
    padb[:npad] = -30.0
    tokp1 = np.zeros((P, 8), np.float32)
    for a in range(8):
        j = a * P + np.arange(P)
        tokp1[:, a] = c * TQ + j // 2 + 1
    corelt = np.zeros((8, 1), np.float32)
    corelt[:c] = 1.0
    return dict(
        xT=np.ascontiguousarray(xc.T),
        xq=np.ascontiguousarray(x[b, qs : qs + TQ]),
        ve=vec, cosk=cosk, sink=sink, padb=padb,
        tokp1=tokp1, corelt=corelt,
        fcT=np.ascontiguousarray(fc_w[c].T.astype(ml_dtypes.bfloat16)),
        projT=np.ascontiguousarray(proj_w[c].T.astype(ml_dtypes.bfloat16)),
        **shared,
    )


def _make_in_maps(inputs):
    return _prep_all(**inputs)


def _prep_all(x, ve, cos, sin, c_q_w, c_k_w, c_v_w, c_proj_w, ve_gate_w,
              router_w, fc_w, proj_w, window_size):
    x = np.asarray(x, np.float32)
    ve = np.asarray(ve, np.float32)
    cos = np.asarray(cos, np.float32)
    sin = np.asarray(sin, np.float32)
    trimask = np.zeros((8 * P, TQ), ml_dtypes.bfloat16)
    kk = np.arange(P)[:, None]
    qq = np.arange(TQ)[None, :]
    for kt in range(4):
        trimask[kt * P : (kt + 1) * P] = (qq <= kk + P * kt).astype(ml_dtypes.bfloat16)
    for i in range(4):
        trimask[(4 + i) * P : (5 + i) * P] = (qq >= kk + P * i).astype(ml_dtypes.bfloat16)
    shared = dict(
        trimask=trimask,
        wqT=np.ascontiguousarray(np.asarray(c_q_w, np.float32).T),
        wkT=np.ascontiguousarray(np.asarray(c_k_w, np.float32).T),
        wvT=np.ascontiguousarray(np.asarray(c_v_w, np.float32).T),
        gateT=np.ascontiguousarray(np.asarray(ve_gate_w, np.float32).T),
        woT=np.ascontiguousarray(np.asarray(c_proj_w, np.float32).T.astype(ml_dtypes.bfloat16)),
        routerT=np.ascontiguousarray(np.asarray(router_w, np.float32).T),
        e2048=(np.arange(8, dtype=np.float32) * CAP).reshape(8, 1),
        iota8=np.broadcast_to(np.arange(8, dtype=np.float32), (P, 8)).copy(),
    )
    fc_w = np.asarray(fc_w, np.float32)
    proj_w = np.asarray(proj_w, np.float32)
    return [
        _prep_core_inputs(c, x, ve, cos, sin, fc_w, proj_w, shared)
        for c in range(NCORES)
    ]


def kernel(**inputs):
    in_maps = _prep_all(**inputs)
    nc = _get_nc()
    global LAST_EXEC_NS, LAST_RESULTS
    res = run_bass_kernel_spmd(nc, in_maps, core_ids=list(range(NCORES)), trace=TRACE)
    LAST_EXEC_NS = res.exec_time_ns
    LAST_RESULTS = res
    out = np.empty((NTOK, C), np.float32)
    rw = np.empty((NTOK, E), np.float32)
    for c in range(NCORES):
        out[c * TQ : (c + 1) * TQ] = res.results[c]["x1o"]
        rw[c * TQ : (c + 1) * TQ] = res.results[c]["rwo"]
    for c in range(NCORES):
        out += res.results[c]["moeo"]
    return out.reshape(B, T, C), rw.reshape(B, T, E)


# revision 25
# speedup vs baseline: 170.2332x; 1.2366x over previous
"""Trainium2 Bass kernel for nn_BlockMoE (attention + top-2 MoE block), 8-core SPMD.

Sharding: attention is T-sharded (core c handles batch c//4, a 512-token chunk
with a 1024-token KV halo); MoE is expert-parallel (core e owns expert e).
Cross-core exchange: AllGather of per-core expert counts, AllGather of xn1
(bf16), ReduceScatter of the dispatch table.  Host only shards inputs /
gathers and sums outputs.
"""
import sys

for _p in ("/opt/trn_rl_repo",):
    if _p not in sys.path:
        sys.path.insert(0, _p)

import numpy as np
import ml_dtypes

import concourse.bass as bass
import concourse.mybir as mybir
import bass_rust as _bass_rust
from concourse.tile import TileContext
from concourse.masks import make_identity
from concourse.bass_utils import run_bass_kernel_spmd

F32 = mybir.dt.float32
F32R = mybir.dt.float32r
BF16 = mybir.dt.bfloat16
I32 = mybir.dt.int32
U32 = mybir.dt.uint32
AF = mybir.ActivationFunctionType
OP = mybir.AluOpType
AX = mybir.AxisListType

P = 128
B, T, C = 2, 2048, 1024
NH, NKV, HD = 16, 8, 64
E, H = 8, 2048
NTOK = B * T
TQ = 512
TK = 1536
KT = TK // P       # 12
QT = TQ // P       # 4
CT = C // P        # 8
HT = H // P        # 16
CAP = 2 * NTOK * 2 // E   # 2048
EPS = float(np.finfo(np.float32).eps)
NCORES = 8
QK_DT = F32R       # dtype for q^T/k^T storage + scores matmul


def _split_waits(nc, max_waits=1):
    """This walrus accepts at most 1 sem-wait per instruction; hoist the rest
    onto nops inserted just before."""
    n_fixed = 0
    for bb in nc.main_func.blocks:
        targets = [
            ins
            for ins in bb.instructions
            if ins.sync_info is not None
            and ins.sync_info.on_wait
            and len(ins.sync_info.on_wait) > max_waits
            and ins.engine != mybir.EngineType.Unassigned
        ]
        for ins in targets:
            waits = list(ins.sync_info.on_wait)
            keep, rest = waits[:max_waits], waits[max_waits:]
            nops = []
            for i in range(0, len(rest), max_waits):
                chunk = rest[i : i + max_waits]
                bi = nc.engines[ins.engine].nop(nofuse=True)
                nop_inst = bi.ins
                for bb2 in nc.main_func.blocks:
                    if nop_inst in bb2.instructions:
                        bb2.instructions.remove(nop_inst)
                nop_inst.sync_info = _bass_rust.SyncInfo(on_wait=chunk, on_update=[])
                nops.append(nop_inst)
                n_fixed += 1
            ins.sync_info = _bass_rust.SyncInfo(
                on_wait=keep, on_update=list(ins.sync_info.on_update or [])
            )
            pos = bb.instructions.index(ins)
            bb.instructions[pos:pos] = nops
    return n_fixed


def _build():
    nc = bass.Bass("TRN2", target_bir_lowering=False)
    dp = nc.declare_dram_parameter
    io = {}
    io["xT_in"] = dp("xT", [C, TK], F32R, isOutput=False)
    io["xq_in"] = dp("xq", [TQ, C], F32, isOutput=False)
    io["ve_in"] = dp("ve", [TK, NKV * HD], F32, isOutput=False)
    io["cosk_in"] = dp("cosk", [TK, 32], F32, isOutput=False)
    io["sink_in"] = dp("sink", [TK, 32], F32, isOutput=False)
    io["padb_in"] = dp("padb", [TK, 1], F32, isOutput=False)
    io["trimask_in"] = dp("trimask", [8 * P, TQ], BF16, isOutput=False)
    io["wqT_in"] = dp("wqT", [C, NH * HD], F32R, isOutput=False)
    io["wkT_in"] = dp("wkT", [C, NKV * HD], F32R, isOutput=False)
    io["wvT_in"] = dp("wvT", [C, NKV * HD], F32R, isOutput=False)
    io["gateT_in"] = dp("gateT", [32, 8], F32R, isOutput=False)
    io["woT_in"] = dp("woT", [C, C], BF16, isOutput=False)
    io["routerT_in"] = dp("routerT", [C, E], F32, isOutput=False)
    io["fcT_in"] = dp("fcT", [C, H], BF16, isOutput=False)
    io["projT_in"] = dp("projT", [H, C], BF16, isOutput=False)
    io["corelt_in"] = dp("corelt", [8, 1], F32, isOutput=False)
    io["e2048_in"] = dp("e2048", [8, 1], F32, isOutput=False)
    io["iota8_in"] = dp("iota8", [P, 8], F32, isOutput=False)
    io["tokp1_in"] = dp("tokp1", [P, 8], F32, isOutput=False)
    io["x1_out"] = dp("x1o", [TQ, C], F32, isOutput=True)
    io["rw_out"] = dp("rwo", [TQ, E], F32, isOutput=True)
    io["moe_out"] = dp("moeo", [NTOK, C], BF16, isOutput=True)

    with TileContext(nc) as tc:
        _program(nc, tc, io)
    _split_waits(nc)
    return nc


def _program(nc, tc, io):
    import contextlib

    ctx = contextlib.ExitStack()
    with ctx:
        const = ctx.enter_context(tc.tile_pool(name="const", bufs=1))
        wpool = ctx.enter_context(tc.tile_pool(name="wpool", bufs=1))
        spool = ctx.enter_context(tc.tile_pool(name="spool", bufs=2))
        hold = ctx.enter_context(tc.tile_pool(name="hold", bufs=1))
        psA = ctx.enter_context(tc.tile_pool(name="psA", bufs=1, space="PSUM"))
        psB = ctx.enter_context(tc.tile_pool(name="psB", bufs=4, space="PSUM"))
        psY = ctx.enter_context(tc.tile_pool(name="psY", bufs=1, space="PSUM"))

        _ctr = [0]

        def pa():
            _ctr[0] += 1
            return psA.tile([P, 1024], F32, tag="A", name=f"psA_{_ctr[0]}")

        def pb(shape=None):
            _ctr[0] += 1
            return psB.tile([P, 512], F32, tag="B", name=f"psB_{_ctr[0]}")

        def pbb():
            _ctr[0] += 1
            return psB.tile([P, 512], BF16, tag="B", name=f"psBb_{_ctr[0]}")

        dram = ctx.enter_context(tc.tile_pool(name="dram", bufs=1, space="DRAM"))

        # ===== constants =====
        ident = const.tile([P, P], F32)
        make_identity(nc, ident[:])
        identb = const.tile([P, P], BF16)
        nc.vector.tensor_copy(identb[:], ident[:])
        ones1b = const.tile([P, 1], BF16)
        nc.vector.memset(ones1b[:], 1.0)
        ones8 = const.tile([8, 1], F32)
        nc.vector.memset(ones8[:], 1.0)
        onesb = const.tile([P, HD], BF16)
        nc.vector.memset(onesb[:], 1.0)
        padb = const.tile([P, KT], F32)
        nc.sync.dma_start(out=padb[:].rearrange("p (a b) -> p a b", b=1), in_=io["padb_in"][:].rearrange("(a p) b -> p a b", p=P))
        iota8 = const.tile([P, 8], F32)
        nc.sync.dma_start(out=iota8[:], in_=io["iota8_in"][:])
        tokp1 = const.tile([P, 8], F32)
        nc.sync.dma_start(out=tokp1[:], in_=io["tokp1_in"][:])
        corelt = const.tile([8, 1], F32)
        nc.sync.dma_start(out=corelt[:], in_=io["corelt_in"][:])
        epsc = const.tile([P, 1], F32)
        nc.vector.memset(epsc[:], EPS)
        e2048 = const.tile([8, 1], F32)
        nc.sync.dma_start(out=e2048[:], in_=io["e2048_in"][:])
        gateT = const.tile([32, 8], F32R)
        nc.sync.dma_start(out=gateT[:], in_=io["gateT_in"][:])
        routerT = const.tile([P, CT * E], F32)
        nc.sync.dma_start(
            out=routerT[:].rearrange("p (a e) -> p a e", e=E), in_=io["routerT_in"][:].rearrange("(a p) e -> p a e", p=P)
        )
        cosk, sink = [], []
        for tt in range(KT):
            ctile = const.tile([P, 32], F32, tag=f"cos{tt}")
            stile = const.tile([P, 32], F32, tag=f"sin{tt}")
            nc.sync.dma_start(out=ctile[:], in_=io["cosk_in"][tt * P : (tt + 1) * P, :])
            nc.sync.dma_start(out=stile[:], in_=io["sink_in"][tt * P : (tt + 1) * P, :])
            cosk.append(ctile)
            sink.append(stile)

        # resident attention out (transposed q/k spilled to DRAM)
        yT = hold.tile([HD, NH * TQ], BF16)
        rcol = hold.tile([P, KT], F32)
        S = hold.tile([8, TQ * 2], BF16)
        vext = [hold.tile([P, NKV, HD + 1], BF16, tag=f"vext{tt}", name=f"vext{tt}") for tt in range(KT)]

        # DRAM scratch
        kT_d = dram.tile([HD, NKV * TK], QK_DT)
        qT_d = dram.tile([HD, NH * TQ], QK_DT)
        xn1b = dram.tile([TQ, C], BF16)
        agx = dram.tile([NTOK, C], BF16, addr_space="Shared")
        cntb = dram.tile([8, 1], F32)
        cntag = dram.tile([NCORES * 8, 1], F32, addr_space="Shared")
        wtmp = dram.tile([TQ * 2, 1], F32)
        ttmp = dram.tile([TQ * 2, 1], F32)
        table = dram.tile([E * CAP, 2], F32)
        tabrs = dram.tile([CAP, 2], F32)

        # weight slots (reused across passes)
        def load_w(name, cols, dtype, tagp):
            tiles = []
            for i in range(CT):
                t = wpool.tile([P, cols], dtype, tag=f"{tagp}{i}")
                nc.sync.dma_start(out=t[:], in_=io[name][i * P : (i + 1) * P, :])
                tiles.append(t)
            return tiles

        def stream_x(tt, tag="xs"):
            xs = []
            for ct in range(CT):
                t = spool.tile([P, P], F32R, tag=f"{tag}{ct}", bufs=2)
                nc.sync.dma_start(
                    out=t[:], in_=io["xT_in"][ct * P : (ct + 1) * P, tt * P : (tt + 1) * P]
                )
                xs.append(t)
            return xs

        def rope_norm(nc, ps_raw, nh, tt, dst, dst_col):
            """psum raw [P, nh*HD] -> rope -> per-head rms -> transpose into
            dst[:, head*W + dst_col*P : ...] (dst width W per head)."""
            rc = rcol[:, tt : tt + 1]
            raw = spool.tile([P, 16, HD], F32, tag="rp_raw")
            r3 = raw[:, 0:nh, :]
            nc.vector.tensor_scalar_mul(r3, ps_raw[:].rearrange("p (h d) -> p h d", h=nh), rc)
            cb = cosk[tt][:].rearrange("p (h d) -> p h d", h=1).to_broadcast([P, nh, 32])
            sb = sink[tt][:].rearrange("p (h d) -> p h d", h=1).to_broadcast([P, nh, 32])
            m1 = spool.tile([P, 16, 32], F32, tag="rp_m1", bufs=1)
            m2 = spool.tile([P, 16, 32], F32, tag="rp_m2", bufs=1)
            hat = spool.tile([P, 16, HD], F32, tag="rp_hn", bufs=1)
            h3 = hat[:, 0:nh, :]
            nc.vector.tensor_tensor(out=m1[:, 0:nh], in0=r3[:, :, 0:32], in1=cb, op=OP.mult)
            nc.vector.tensor_tensor(out=m2[:, 0:nh], in0=r3[:, :, 32:64], in1=sb, op=OP.mult)
            nc.vector.tensor_tensor(out=h3[:, :, 0:32], in0=m1[:, 0:nh], in1=m2[:, 0:nh], op=OP.add)
            nc.vector.tensor_tensor(out=m1[:, 0:nh], in0=r3[:, :, 32:64], in1=cb, op=OP.mult)
            nc.vector.tensor_tensor(out=m2[:, 0:nh], in0=r3[:, :, 0:32], in1=sb, op=OP.mult)
            nc.vector.tensor_tensor(out=h3[:, :, 32:64], in0=m1[:, 0:nh], in1=m2[:, 0:nh], op=OP.subtract)
            sq = spool.tile([P, 16, HD], F32, tag="rp_raw", name="rp_sqv")
            nc.vector.tensor_tensor(out=sq[:, 0:nh], in0=h3, in1=h3, op=OP.mult)
            ssum = spool.tile([P, 16], F32, tag="rp_ss")
            nc.vector.tensor_reduce(ssum[:, 0:nh], sq[:, 0:nh], axis=AX.X, op=OP.add)
            rh = spool.tile([P, 16], F32, tag="rp_rh")
            nc.scalar.activation(rh[:, 0:nh], ssum[:, 0:nh], AF.Sqrt, bias=epsc[:], scale=1.0 / HD)
            nc.vector.reciprocal(rh[:, 0:nh], rh[:, 0:nh])
            nc.vector.tensor_tensor(
                out=h3, in0=h3, in1=rh[:, 0:nh].to_broadcast([P, nh, HD]), op=OP.mult
            )
            asm = spool.tile([HD, 16 * P], QK_DT, tag="rp_asm", bufs=1)
            for h in range(nh):
                pt = pb()
                nc.tensor.transpose(pt[0:HD, 0:P], hat[:, h, :], ident[:])
                nc.vector.tensor_copy(asm[:, h * P : (h + 1) * P], pt[0:HD, 0:P])
            nc.sync.dma_start(
                out=dst[:].rearrange("a (h w) -> a h w", h=nh)[:, :, dst_col * P : (dst_col + 1) * P],
                in_=asm[:, 0 : nh * P].rearrange("a (h w) -> a h w", h=nh),
            )

        # ================= pass K+V (+ rms1 fused) =================
        wkv = []
        for i in range(CT):
            t = wpool.tile([P, 2 * NKV * HD], F32R, tag=f"ws{i}", name=f"wkv{i}")
            nc.sync.dma_start(out=t[:, 0 : NKV * HD], in_=io["wkT_in"][i * P : (i + 1) * P, :])
            nc.sync.dma_start(out=t[:, NKV * HD :], in_=io["wvT_in"][i * P : (i + 1) * P, :])
            wkv.append(t)
        for tt in range(KT):
            xs = stream_x(tt)
            # rms1 for this token tile
            ss = pb()
            for ct in range(CT):
                sq = spool.tile([P, P], BF16, tag="sq1", bufs=2)
                nc.scalar.activation(sq[:], xs[ct][:].bitcast(F32), AF.Square)
                nc.tensor.matmul(ss[0:1, 0:P], ones1b[:], sq[:], start=(ct == 0), stop=(ct == CT - 1))
            sq1 = spool.tile([1, P], F32, tag="sq1r", bufs=2)
            nc.vector.tensor_copy(sq1[:], ss[0:1, 0:P])
            pt = pb()
            nc.tensor.transpose(pt[0:P, 0:1], sq1[:], ident[0:1, 0:1])
            nc.scalar.activation(rcol[:, tt : tt + 1], pt[0:P, 0:1], AF.Sqrt, bias=epsc[:], scale=1.0 / C)
            nc.vector.reciprocal(rcol[:, tt : tt + 1], rcol[:, tt : tt + 1])
            # k
            ps_k = pb()
            for ct in range(CT):
                nc.tensor.matmul(ps_k[:], xs[ct][:], wkv[ct][:, 0 : NKV * HD], start=(ct == 0), stop=(ct == CT - 1))
            rope_norm(nc, ps_k, NKV, tt, kT_d, tt)
            rc = rcol[:, tt : tt + 1]
            ps_v = pb()
            for ct in range(CT):
                nc.tensor.matmul(ps_v[:], xs[ct][:], wkv[ct][:, NKV * HD :], start=(ct == 0), stop=(ct == CT - 1))
            ps_g = pb()
            nc.tensor.matmul(ps_g[0:P, 0:8], xs[0][0:32, :], gateT[:], start=True, stop=True)
            ga = spool.tile([P, NKV], F32, tag="ga")
            nc.scalar.activation(ga[:], ps_g[0:P, 0:8], AF.Sigmoid, scale=rc)
            vtmp = spool.tile([P, NKV * HD], F32, tag="vtmp")
            nc.vector.tensor_scalar_mul(vtmp[:], ps_v[:], rc)
            vet = spool.tile([P, NKV * HD], F32, tag="vet", bufs=2)
            nc.sync.dma_start(out=vet[:], in_=io["ve_in"][tt * P : (tt + 1) * P, :])
            nc.vector.tensor_tensor(
                out=vet[:].rearrange("p (h d) -> p h d", h=NKV),
                in0=vet[:].rearrange("p (h d) -> p h d", h=NKV),
                in1=ga[:].to_broadcast([P, NKV, HD]),
                op=OP.mult,
            )
            nc.vector.scalar_tensor_tensor(
                out=vtmp[:], in0=vet[:], scalar=2.0, in1=vtmp[:], op0=OP.mult, op1=OP.add
            )
            nc.vector.tensor_copy(vext[tt][:, :, 0:HD], vtmp[:].rearrange("p (h d) -> p h d", h=NKV))
            nc.vector.memset(vext[tt][:, :, HD : HD + 1], 1.0)

        # ================= pass Q =================
        wq = load_w("wqT_in", NH * HD, F32R, "ws")
        for tt in range(8, KT):
            xs = stream_x(tt)
            ps_q = pa()
            for half in range(2):
                for ct in range(CT):
                    nc.tensor.matmul(
                        ps_q[:, half * 512 : (half + 1) * 512],
                        xs[ct][:],
                        wq[ct][:, half * 512 : (half + 1) * 512],
                        start=(ct == 0),
                        stop=(ct == CT - 1),
                    )
            rope_norm(nc, ps_q, NH, tt, qT_d, tt - 8)

        # ================= A4: scores/softmax/pv =================
        yTv = yT[:].rearrange("a (h w) -> a h w", h=NH)
        for h in range(NKV):
            kTh = spool.tile([HD, TK], QK_DT, tag="kTh", bufs=1)
            nc.sync.dma_start(out=kTh[:], in_=kT_d[:].rearrange("a (h w) -> a h w", h=NKV)[:, h, :])
            for qh in (2 * h, 2 * h + 1):
                qTh = spool.tile([HD, TQ], QK_DT, tag="qTh", bufs=1)
                nc.sync.dma_start(out=qTh[:], in_=qT_d[:].rearrange("a (h w) -> a h w", h=NH)[:, qh, :])
                ps_y = psY.tile([HD + 1, TQ], F32, tag="Y")
                for kt in range(KT):
                    ps_s = pb()
                    nc.tensor.matmul(
                        ps_s[:], kTh[:, kt * P : (kt + 1) * P], qTh[:],
                        start=True, stop=True,
                    )
                    pT = spool.tile([P, TQ], BF16, tag="pT", bufs=2)
                    nc.scalar.activation(
                        pT[:], ps_s[:], AF.Exp, bias=padb[:, kt : kt + 1], scale=0.125
                    )
                    if kt < 4:
                        nc.gpsimd.affine_select(
                            pT[:], pT[:], pattern=[[-1, TQ]], compare_op=OP.is_ge,
                            fill=0.0, base=P * kt, channel_multiplier=1)
                    elif kt >= 8:
                        nc.gpsimd.affine_select(
                            pT[:], pT[:], pattern=[[1, TQ]], compare_op=OP.is_ge,
                            fill=0.0, base=-P * (kt - 8), channel_multiplier=-1)
                    nc.tensor.matmul(
                        ps_y[:], vext[kt][:, h, :], pT[:], start=(kt == 0), stop=(kt == KT - 1)
                    )
                zrow = spool.tile([P, TQ], F32, tag="vtmp")
                nc.vector.reciprocal(zrow[HD : HD + 1, :], ps_y[HD : HD + 1, :])
                bcs = spool.tile([P, TQ], BF16, tag="bcs")
                nc.vector.tensor_copy(bcs[HD : HD + 1, :], zrow[HD : HD + 1, :])
                ps_b = pb()
                nc.tensor.matmul(
                    ps_b[0:HD, :], onesb[HD : HD + 1, :], bcs[HD : HD + 1, :],
                    start=True, stop=True,
                )
                nc.vector.tensor_copy(bcs[0:HD, :], ps_b[0:HD, :])
                nc.vector.tensor_tensor(
                    out=yTv[:, qh, :], in0=ps_y[0:HD, :], in1=bcs[0:HD, :], op=OP.mult
                )

        # ================= A5/A6: proj+residual+rms2+router =================
        woT = []
        for i in range(NH):
            wt = wpool.tile([HD, C], BF16, tag=f"wo{i}", name=f"woT{i}")
            nc.sync.dma_start(out=wt[:], in_=io["woT_in"][i * HD : (i + 1) * HD, :])
            woT.append(wt)
        for m in range(QT):
            ps_o = pa()
            for n in range(2):
                for qh in range(NH):
                    nc.tensor.matmul(
                        ps_o[:, n * 512 : (n + 1) * 512],
                        yTv[:, qh, m * P : (m + 1) * P],
                        woT[qh][:, n * 512 : (n + 1) * 512],
                        start=(qh == 0),
                        stop=(qh == NH - 1),
                    )
            xqt = spool.tile([P, C], F32, tag="rp_raw", bufs=2)
            nc.sync.dma_start(out=xqt[:], in_=io["xq_in"][m * P : (m + 1) * P, :])
            x1m = spool.tile([P, C], F32, tag="x1m", bufs=1)
            nc.vector.tensor_tensor(out=x1m[:], in0=ps_o[:], in1=xqt[:], op=OP.add)
            nc.sync.dma_start(out=io["x1_out"][m * P : (m + 1) * P, :], in_=x1m[:])
            sq = spool.tile([P, C], F32, tag="rp_hn", bufs=1)
            nc.vector.tensor_tensor(out=sq[:], in0=x1m[:], in1=x1m[:], op=OP.mult)
            ssum = spool.tile([P, 1], F32, tag="ss_a6")
            nc.vector.tensor_reduce(ssum[:], sq[:], axis=AX.X, op=OP.add)
            r1 = spool.tile([P, 1], F32, tag="r1_a6")
            nc.scalar.activation(r1[:], ssum[:], AF.Sqrt, bias=epsc[:], scale=1.0 / C)
            nc.vector.reciprocal(r1[:], r1[:])
            xn1 = spool.tile([P, C], F32, tag="rp_sq2", bufs=2)
            nc.vector.tensor_scalar_mul(xn1[:], x1m[:], r1[:])
            xn1c = spool.tile([P, C], BF16, tag="xn1c", bufs=1)
            nc.vector.tensor_copy(xn1c[:], xn1[:])
            nc.sync.dma_start(out=xn1b[m * P : (m + 1) * P, :], in_=xn1c[:])
            # router (plain f32 matmuls, exact)
            ps_r = pb()
            for ct in range(CT):
                ptr = pb()
                nc.tensor.transpose(ptr[:, 0:P], xn1[:, ct * P : (ct + 1) * P], ident[:])
                xn1T = spool.tile([P, P], F32, tag="xn1T", bufs=2)
                nc.vector.tensor_copy(xn1T[:], ptr[:, 0:P])
                nc.tensor.matmul(
                    ps_r[0:P, 0:E], xn1T[:], routerT[:, ct * E : (ct + 1) * E],
                    start=(ct == 0), stop=(ct == CT - 1),
                )
            er = spool.tile([P, E], F32, tag="er")
            nc.scalar.activation(er[:], ps_r[0:P, 0:E], AF.Exp)
            es = spool.tile([P, 1], F32, tag="es")
            nc.vector.tensor_reduce(es[:], er[:], axis=AX.X, op=OP.add)
            nc.vector.reciprocal(es[:], es[:])
            rw = spool.tile([P, E], F32, tag="rw")
            nc.vector.tensor_scalar_mul(rw[:], er[:], es[:])
            nc.sync.dma_start(out=io["rw_out"][m * P : (m + 1) * P, :], in_=rw[:])
            tv = spool.tile([P, 8], F32, tag="tv")
            ti = spool.tile([P, 8], U32, tag="ti")
            nc.vector.max_with_indices(tv[:], ti[:], rw[:])
            tif = spool.tile([P, 8], F32, tag="tif")
            nc.vector.tensor_copy(tif[:], ti[:])
            den = spool.tile([P, 1], F32, tag="den")
            nc.vector.tensor_tensor(out=den[:], in0=tv[:, 0:1], in1=tv[:, 1:2], op=OP.add)
            nc.vector.tensor_scalar_add(den[:], den[:], 1e-10)
            nc.vector.reciprocal(den[:], den[:])
            swp = spool.tile([P, 2], F32, tag="swp")
            nc.vector.tensor_scalar_mul(swp[:], tv[:, 0:2], den[:])
            nc.sync.dma_start(
                out=wtmp[m * 256 : (m + 1) * 256, :].rearrange("(p c) b -> p c b", c=2),
                in_=swp[:].rearrange("p (c b) -> p c b", b=1),
            )
            for k in range(2):
                oh = spool.tile([P, 8], F32, tag="oh")
                nc.vector.tensor_tensor(
                    out=oh[:], in0=tif[:, k : k + 1].to_broadcast([P, 8]), in1=iota8[:],
                    op=OP.is_equal,
                )
                pso = pb()
                nc.tensor.transpose(pso[0:8, 0:P], oh[:], ident[:])
                nc.vector.tensor_copy(
                    S[:].rearrange("e (x c) -> e x c", c=2)[:, m * P : (m + 1) * P, k],
                    pso[0:8, 0:P],
                )

        # ================= A7: routing =================
        zc8 = const.tile([8, 1], F32)
        nc.vector.memset(zc8[:], 0.0)
        incl = hold.tile([8, TQ * 2], F32)
        nc.vector.tensor_tensor_scan(
            incl[:], S[:], zc8[:].to_broadcast([8, TQ * 2]), 0.0, op0=OP.add, op1=OP.add
        )
        cnt = spool.tile([8, 1], F32, tag="cnt")
        nc.vector.tensor_copy(cnt[:], incl[:, TQ * 2 - 1 : TQ * 2])
        nc.vector.tensor_tensor(out=incl[:], in0=incl[:], in1=S[:], op=OP.subtract)
        nc.sync.dma_start(out=cntb[:], in_=cnt[:])
        nc.gpsimd.collective_compute(
            "AllGather", OP.bypass, ins=[cntb[:].opt()], outs=[cntag[:].opt()],
            replica_groups=[list(range(NCORES))],
        )
        nc.gpsimd.collective_compute(
            "AllGather", OP.bypass, ins=[xn1b[:].opt()], outs=[agx[:].opt()],
            replica_groups=[list(range(NCORES))],
        )
        cnts = spool.tile([8, 8], F32, tag="cnts")
        nc.sync.dma_start(out=cnts[:].rearrange("c (e b) -> c e b", b=1), in_=cntag[:].rearrange("(c e) b -> c e b", e=8))
        ps_off = pb()
        nc.tensor.matmul(ps_off[0:8, 0:1], cnts[:], corelt[:], start=True, stop=True)
        off = spool.tile([8, 1], F32, tag="off")
        nc.vector.tensor_copy(off[:], ps_off[0:8, 0:1])
        nc.vector.tensor_scalar(incl[:], incl[:], off[:], None, op0=OP.add)
        ovf = spool.tile([8, TQ * 2], BF16, tag="vtmp", name="ovf")
        nc.vector.tensor_scalar(ovf[:], incl[:], float(CAP), None, op0=OP.is_ge)
        nc.vector.tensor_scalar(incl[:], incl[:], e2048[:], None, op0=OP.add)
        nc.vector.scalar_tensor_tensor(
            out=incl[:], in0=ovf[:], scalar=1e6, in1=incl[:], op0=OP.mult, op1=OP.add
        )
        nc.vector.tensor_tensor(out=incl[:], in0=incl[:], in1=S[:], op=OP.mult)
        for n in range(2):
            ps_t = pb()
            nc.tensor.matmul(
                ps_t[0:1, :], ones8[:], incl[:, n * 512 : (n + 1) * 512], start=True, stop=True
            )
            trh = spool.tile([1, 512], F32, tag="sq1r", name=f"trh{n}", bufs=2)
            nc.vector.tensor_copy(trh[:], ps_t[0:1, :])
            nc.sync.dma_start(
                out=ttmp[n * 512 : (n + 1) * 512, :].rearrange("a b -> b a"), in_=trh[:]
            )
        tgt_sb = spool.tile([P, 8], F32, tag="tgt_sb")
        nc.sync.dma_start(out=tgt_sb[:].rearrange("p (a b) -> p a b", b=1), in_=ttmp[:].rearrange("(a p) b -> p a b", p=P))
        w_ent = spool.tile([P, 8], F32, tag="w_ent")
        nc.sync.dma_start(out=w_ent[:].rearrange("p (a b) -> p a b", b=1), in_=wtmp[:].rearrange("(a p) b -> p a b", p=P))
        iT = spool.tile([P, 8], I32, tag="iT")
        nc.vector.tensor_copy(iT[:], tgt_sb[:])
        zz = spool.tile([P, 256], F32, tag="zz", bufs=1)
        nc.vector.memset(zz[:], 0.0)
        nc.sync.dma_start(out=table[:].rearrange("(p a) c -> p a c", p=P), in_=zz[:].rearrange("p (a c) -> p a c", c=2))
        for a in range(8):
            pay = spool.tile([P, 2], F32, tag="pay", bufs=2)
            nc.vector.tensor_copy(pay[:, 0:1], tokp1[:, a : a + 1])
            nc.vector.tensor_copy(pay[:, 1:2], w_ent[:, a : a + 1])
            nc.gpsimd.indirect_dma_start(
                out=table[:, :],
                out_offset=bass.IndirectOffsetOnAxis(ap=iT[:, a : a + 1], axis=0),
                in_=pay[:],
                in_offset=None,
                bounds_check=E * CAP - 1,
                oob_is_err=False,
            )
        nc.gpsimd.collective_compute(
            "ReduceScatter", OP.add, ins=[table[:].opt()], outs=[tabrs[:].opt()],
            replica_groups=[list(range(NCORES))],
        )

        # ================= A8: expert FFN =================
        fcT = load_w("fcT_in", H, BF16, "ws")
        projT = []
        for i in range(HT):
            t = wpool.tile([P, C], BF16, tag=f"wo{i}", name=f"projT{i}")
            nc.sync.dma_start(out=t[:], in_=io["projT_in"][i * P : (i + 1) * P, :])
            projT.append(t)

        for b4 in range(CAP // 512):
            tw, gidx = [], []
            bufT = [spool.tile([P, 512], BF16, tag=f"bufT{ct}", name=f"bufT{ct}_{b4}", bufs=1) for ct in range(CT)]
            for rsub in range(4):
                twt = spool.tile([P, 2], F32, tag="twt", bufs=4)
                nc.sync.dma_start(
                    out=twt[:], in_=tabrs[b4 * 512 + rsub * P : b4 * 512 + (rsub + 1) * P, :]
                )
                tw.append(twt)
                gif = spool.tile([P, 1], F32, tag="gif", bufs=4)
                nc.vector.tensor_scalar_add(gif[:], twt[:, 0:1], -1.0)
                gi = spool.tile([P, 1], I32, tag="gi", bufs=4)
                nc.vector.tensor_copy(gi[:], gif[:])
                gidx.append(gi)
                buf = spool.tile([P, C], BF16, tag="buf", bufs=1)
                nc.vector.memset(buf[:], 0.0)
                nc.gpsimd.indirect_dma_start(
                    out=buf[:], out_offset=None, in_=agx[:],
                    in_offset=bass.IndirectOffsetOnAxis(ap=gi[:, 0:1], axis=0),
                    bounds_check=NTOK - 1, oob_is_err=False,
                )
                for ct in range(CT):
                    ptb = pbb()
                    nc.tensor.transpose(ptb[:, 0:P], buf[:, ct * P : (ct + 1) * P], identb[:])
                    nc.vector.tensor_copy(bufT[ct][:, rsub * P : (rsub + 1) * P], ptb[:, 0:P])
            hT = []
            for hm in range(HT):
                ps_h = pb()
                for ct in range(CT):
                    nc.tensor.matmul(
                        ps_h[:], fcT[ct][:, hm * P : (hm + 1) * P], bufT[ct][:],
                        start=(ct == 0), stop=(ct == CT - 1),
                    )
                hR = spool.tile([P, 512], BF16, tag="hR", bufs=2)
                nc.scalar.activation(hR[:], ps_h[:], AF.Relu)
                ht = spool.tile([P, 512], BF16, tag=f"hT{hm}", bufs=1)
                nc.vector.tensor_tensor(out=ht[:], in0=hR[:], in1=hR[:], op=OP.mult)
                hT.append(ht)
            for rsub in range(4):
                ps_p = pa()
                for n in range(2):
                    for hm in range(HT):
                        nc.tensor.matmul(
                            ps_p[:, n * 512 : (n + 1) * 512],
                            hT[hm][:, rsub * P : (rsub + 1) * P],
                            projT[hm][:, n * 512 : (n + 1) * 512],
                            start=(hm == 0),
                            stop=(hm == HT - 1),
                        )
                bo = spool.tile([P, C], BF16, tag="rp_raw", bufs=2, name=f"bo{b4}_{rsub}")
                nc.vector.tensor_scalar_mul(bo[:], ps_p[:], tw[rsub][:, 1:2])
                nc.gpsimd.indirect_dma_start(
                    out=io["moe_out"][:, :],
                    out_offset=bass.IndirectOffsetOnAxis(ap=gidx[rsub][:, 0:1], axis=0),
                    in_=bo[:],
                    in_offset=None,
                    bounds_check=NTOK - 1,
                    oob_is_err=False,
                )


# ======================= host side =======================
_NC_CACHE = {}
TRACE = False
LAST_EXEC_NS = None
LAST_RESULTS = None


def _get_nc():
    if "nc" not in _NC_CACHE:
        _NC_CACHE["nc"] = _build()
    return _NC_CACHE["nc"]


def _prep_core_inputs(c, x, ve, cos, sin, fc_w, proj_w, shared):
    b, ch = c // 4, c % 4
    qs = ch * TQ
    ks0 = qs - 1024
    npad = max(0, -ks0)
    xc = np.zeros((TK, C), np.float32)
    xc[npad:] = x[b, max(ks0, 0) : qs + TQ]
    vec = np.zeros((TK, NKV * HD), np.float32)
    vec[npad:] = ve[b, max(ks0, 0) : qs + TQ]
    cosk = np.zeros((TK, 32), np.float32)
    sink = np.zeros((TK, 32), np.float32)
    cosk[npad:] = cos[0, max(ks0, 0) : qs + TQ, 0]
    sink[npad:] = sin[0, max(ks0, 0) : qs + TQ, 0]
    padb = np.zeros((TK, 1), np.float32)
    padb[:npad] = -30.0
    tokp1 = np.zeros((P, 8), np.float32)
    for a in range(8):
        j = a * P + np.arange(P)
        tokp1[:, a] = c * TQ + j // 2 + 1
    corelt = np.zeros((8, 1), np.float32)
    corelt[:c] = 1.0
    return dict(
        xT=np.ascontiguousarray(xc.T),
        xq=np.ascontiguousarray(x[b, qs : qs + TQ]),
        ve=vec, cosk=cosk, sink=sink, padb=padb,
        tokp1=tokp1, corelt=corelt,
        fcT=np.ascontiguousarray(fc_w[c].T.astype(ml_dtypes.bfloat16)),
        projT=np.ascontiguousarray(proj_w[c].T.astype(ml_dtypes.bfloat16)),
        **shared,
    )


def _make_in_maps(inputs):
    return _prep_all(**inputs)


def _prep_all(x, ve, cos, sin, c_q_w, c_k_w, c_v_w, c_proj_w, ve_gate_w,
              router_w, fc_w, proj_w, window_size):
    x = np.asarray(x, np.float32)
    ve = np.asarray(ve, np.float32)
    cos = np.asarray(cos, np.float32)
    sin = np.asarray(sin, np.float32)
    trimask = np.zeros((8 * P, TQ), ml_dtypes.bfloat16)
    kk = np.arange(P)[:, None]
    qq = np.arange(TQ)[None, :]
    for kt in range(4):
        trimask[kt * P : (kt + 1) * P] = (qq <= kk + P * kt).astype(ml_dtypes.bfloat16)
    for i in range(4):
        trimask[(4 + i) * P : (5 + i) * P] = (qq >= kk + P * i).astype(ml_dtypes.bfloat16)
    shared = dict(
        trimask=trimask,
        wqT=np.ascontiguousarray(np.asarray(c_q_w, np.float32).T),
        wkT=np.ascontiguousarray(np.asarray(c_k_w, np.float32).T),
        wvT=np.ascontiguousarray(np.asarray(c_v_w, np.float32).T),
        gateT=np.ascontiguousarray(np.asarray(ve_gate_w, np.float32).T),
        woT=np.ascontiguousarray(np.asarray(c_proj_w, np.float32).T.astype(ml_dtypes.bfloat16)),
        routerT=np.ascontiguousarray(np.asarray(router_w, np.float32).T),
        e2048=(np.arange(8, dtype=np.float32) * CAP).reshape(8, 1),
        iota8=np.broadcast_to(np.arange(8, dtype=np.float32), (P, 8)).copy(),
    )
    fc_w = np.asarray(fc_w, np.float32)
    proj_w = np.asarray(proj_w, np.float32)
    return [
        _prep_core_inputs(c, x, ve, cos, sin, fc_w, proj_w, shared)
        for c in range(NCORES)
    ]


def kernel(**inputs):
    in_maps = _prep_all(**inputs)
    nc = _get_nc()
    global LAST_EXEC_NS, LAST_RESULTS
    res = run_bass_kernel_spmd(nc, in_maps, core_ids=list(range(NCORES)), trace=TRACE)
    LAST_EXEC_NS = res.exec_time_ns
    LAST_RESULTS = res
    out = np.empty((NTOK, C), np.float32)
    rw = np.empty((NTOK, E), np.float32)
    for c in range(NCORES):
        out[c * TQ : (c + 1) * TQ] = res.results[c]["x1o"]
        rw[c * TQ : (c + 1) * TQ] = res.results[c]["rwo"]
    for c in range(NCORES):
        out += res.results[c]["moeo"].astype(np.float32)
    return out.reshape(B, T, C), rw.reshape(B, T, E)
